# revision 1
# baseline (speedup 1.0000x reference)
"""SAN Bottleneck (pairwise self-attention) Trainium2 kernel.

Sharding: 8 cores = 2 batches x 4 row-blocks of 14 rows (H=56). Each core
receives a reflect-padded input slice (20 rows x 62 cols), so the 7x7
unfold needs no runtime halo exchange and no edge special-casing.

Per-core pipeline (all batchnorms folded into per-channel scale/bias on host):
  bn1+relu -> x1/x2/x3 1x1 convs (fp32r matmuls)
  feat = relu(x1 - shifted-window(x2))            (fp16, DVE/GPSIMD)
  mm1 66->64 (fp16) -> relu+bias (ACT, PSUM evac)
  mm2 64->128 with 4x-replicated head weights -> exp+bias (ACT)
  softmax normalizer + aggregation: shifted-window products (DVE) and
  in-place pairwise tree adds over the 49 taps (DVE+GPSIMD)
  bn2+relu -> wc conv (fp32r) + bias + identity residual.

The x3/aggregation channels are permuted host-side (s-split: tile t,
partition p <-> channel 8*(p//4)+4t+(p%4)) so one 4x-replicated exp tensor
serves both 128-channel tiles without any partition broadcast. The position
branch (batch independent) is precomputed on host as relu(bn(subp)) and DMA'd
into feat rows 64:65.
"""

import numpy as np
import ml_dtypes

bf16_np = ml_dtypes.bfloat16

K = 7
PAD = 3
EPS = 1e-5
B, C, H, W = 2, 256, 56, 56
RB = 14              # rows per core
NQ = RB * W          # 784
ROWS = RB + 2 * PAD  # 20
WP = W + 2 * PAD     # 62
K2 = K * K
CHUNKS = [(0, 4), (4, 4), (8, 3), (11, 3)]

_BUILD_CACHE = {}


def _perm_channels():
    perm = np.zeros(256, np.int64)
    for t in range(2):
        for p in range(128):
            perm[t * 128 + p] = 8 * (p // 4) + 4 * t + (p % 4)
    return perm


def _build_program():
    if "nc" in _BUILD_CACHE:
        return _BUILD_CACHE["nc"]
    import concourse.bass as bass
    import concourse.bacc as bacc
    import concourse.tile as tile
    import concourse.mybir as mybir
    from contextlib import ExitStack

    f32 = mybir.dt.float32
    f32r = mybir.dt.float32r
    f16 = mybir.dt.float16
    bf16 = mybir.dt.bfloat16
    Alu = mybir.AluOpType
    Act = mybir.ActivationFunctionType

    nc = bacc.Bacc("TRN2", target_bir_lowering=False, num_devices=8)

    xp_d = nc.dram_tensor("xp", [2, 128, ROWS, WP], f32, kind="ExternalInput")
    rsubp_d = nc.dram_tensor("rsubp", [2, K2, NQ], f16, kind="ExternalInput")
    w1T_d = nc.dram_tensor("w1T", [2, 128, 64], bf16, kind="ExternalInput")
    w2T_d = nc.dram_tensor("w2T", [2, 128, 64], bf16, kind="ExternalInput")
    w3T_d = nc.dram_tensor("w3T", [2, 128, 2, 128], bf16, kind="ExternalInput")
    wcT_d = nc.dram_tensor("wcT", [2, 128, 2, 128], bf16, kind="ExternalInput")
    cw1T_d = nc.dram_tensor("cw1T", [66, 64], f16, kind="ExternalInput")
    cw2T_d = nc.dram_tensor("cw2T", [64, 128], f16, kind="ExternalInput")
    scal_d = nc.dram_tensor("scal", [128, 14], f32, kind="ExternalInput")
    y_d = nc.dram_tensor("y", [2, 128, RB, W], f32, kind="ExternalOutput")

    def win_ap(base, elem_off, ndj, nr):
        # [P][ndj dj (stride 2)][nr rows (stride WP)][56 cols] into a flat
        # padded [P, ROWS*WP] tile
        return bass.AP(
            tensor=base.tensor,
            offset=base.offset + elem_off,
            ap=[base.ap[0], [2, ndj], [WP, nr], [1, W]],
        )

    def kq_ap(base3, k0, ndj, nqc, nr):
        # [P][ndj (stride 2*nqc)][nr][56] into a [P, 49, nqc] tile at tap k0
        return bass.AP(
            tensor=base3.tensor,
            offset=base3.offset + k0 * nqc,
            ap=[base3.ap[0], [2 * nqc, ndj], [W, nr], [1, W]],
        )

    with tile.TileContext(nc) as tc, ExitStack() as stack:
        consts = stack.enter_context(tc.tile_pool(name="consts", bufs=1))
        xpp = stack.enter_context(tc.tile_pool(name="xpp", bufs=1))
        headsb = stack.enter_context(tc.tile_pool(name="headsb", bufs=1))

        w1s = consts.tile([128, 2, 64], bf16, tag="w1s")
        w2s = consts.tile([128, 2, 64], bf16, tag="w2s")
        w3s = consts.tile([128, 2, 2, 128], bf16, tag="w3s")
        wcs = consts.tile([128, 2, 2, 128], bf16, tag="wcs")
        cw1s = consts.tile([66, 64], f16, tag="cw1s")
        cw2s = consts.tile([64, 128], f16, tag="cw2s")
        scals = consts.tile([128, 14], f32, tag="scals")
        for kt in range(2):
            nc.sync.dma_start(out=w1s[:, kt, :], in_=w1T_d[kt])
            nc.sync.dma_start(out=w2s[:, kt, :], in_=w2T_d[kt])
            nc.sync.dma_start(out=w3s[:, kt, :, :], in_=w3T_d[kt])
            nc.sync.dma_start(out=wcs[:, kt, :, :], in_=wcT_d[kt])
        nc.sync.dma_start(out=cw1s[:], in_=cw1T_d[:])
        nc.sync.dma_start(out=cw2s[:], in_=cw2T_d[:])
        nc.sync.dma_start(out=scals[:], in_=scal_d[:])

        a1 = [scals[:, 0:1], scals[:, 1:2]]
        b1f = [scals[:, 2:3], scals[:, 3:4]]
        b1p = scals[0:64, 4:5]
        b2p = scals[0:64, 5:6]
        b2f = scals[0:64, 6:7]
        cb2r = scals[:, 7:8]
        a3p = [scals[:, 8:9], scals[:, 9:10]]
        b3fp = [scals[:, 10:11], scals[:, 11:12]]
        bcb = [scals[:, 12:13], scals[:, 13:14]]

        xps = [xpp.tile([128, ROWS, WP], f32, tag=f"xp{t}", name=f"xp{t}") for t in range(2)]
        for t in range(2):
            nc.sync.dma_start(out=xps[t][:], in_=xp_d[t])
        obn = [headsb.tile([128, ROWS * WP], bf16, tag=f"obn{t}", name=f"obn{t}") for t in range(2)]
        for t in range(2):
            nc.scalar.activation(
                out=obn[t][:],
                in_=xps[t][:].rearrange("p r w -> p (r w)"),
                func=Act.Relu, bias=b1f[t], scale=a1[t])

        x1s = headsb.tile([64, RB, W], f16, tag="x1s")
        x2p = headsb.tile([64, ROWS * WP], f16, tag="x2p")
        x2sh = headsb.tile([64, ROWS * WP], f16, tag="x2sh")
        x3ps = headsb.tile([128, 2, ROWS * WP], f16, tag="x3ps")
        x3shs = headsb.tile([128, 2, ROWS * WP], f16, tag="x3shs")

        with tc.tile_pool(name="pshead", bufs=4, space="PSUM") as pshead:
            ccuts = [(0, 416), (416, 416), (832, 408)]
            for (o0, n) in ccuts:
                ps = pshead.tile([64, 416], f32, tag="ps64")
                for kt in range(2):
                    nc.tensor.matmul(
                        ps[:, :n], w2s[:, kt, :],
                        obn[kt][:, o0:o0 + n],
                        start=(kt == 0), stop=(kt == 1))
                nc.scalar.activation(out=x2p[:, o0:o0 + n], in_=ps[:, :n],
                                     func=Act.Identity, bias=b2p, scale=1.0)
            for half in range(2):
                ps = pshead.tile([64, 416], f32, tag="ps64")
                for kt in range(2):
                    rhs = obn[kt][:].rearrange("p (r w) -> p r w", w=WP)[
                        :, 3 + 7 * half:3 + 7 * (half + 1), 3:3 + W]
                    nc.tensor.matmul(ps[:, :392], w1s[:, kt, :],
                                     rhs,
                                     start=(kt == 0), stop=(kt == 1))
                nc.scalar.activation(
                    out=x1s[:, 7 * half:7 * (half + 1), :],
                    in_=ps[:, :392].rearrange("p (r w) -> p r w", w=W),
                    func=Act.Identity, bias=b1p, scale=1.0)
            for ot in range(2):
                for (o0, n) in ccuts:
                    ps = pshead.tile([128, 416], f32, tag="ps128")
                    for kt in range(2):
                        nc.tensor.matmul(
                            ps[:, :n], w3s[:, kt, ot, :],
                            obn[kt][:, o0:o0 + n],
                            start=(kt == 0), stop=(kt == 1))
                    nc.scalar.activation(out=x3ps[:, ot, o0:o0 + n],
                                         in_=ps[:, :n], func=Act.Copy)
            for (s0, s1) in ((0, 414), (414, 830), (830, 1238)):
                nc.vector.tensor_copy(out=x2sh[:, s0:s1], in_=x2p[:, s0 + 1:s1 + 1])
            nc.vector.tensor_copy(out=x3shs[:, :, 0:1238],
                                  in_=x3ps[:, :, 1:1239])

        featp = stack.enter_context(tc.tile_pool(name="featp", bufs=2))
        h2p = stack.enter_context(tc.tile_pool(name="h2p", bufs=1))
        e4p = stack.enter_context(tc.tile_pool(name="e4p", bufs=2))
        prodp = stack.enter_context(tc.tile_pool(name="prodp", bufs=1))
        smallp = stack.enter_context(tc.tile_pool(name="smallp", bufs=2))
        zscp = stack.enter_context(tc.tile_pool(name="zscp", bufs=1))
        ps1p = stack.enter_context(tc.tile_pool(name="ps1p", bufs=2, space="PSUM"))
        ps2p = stack.enter_context(tc.tile_pool(name="ps2p", bufs=2, space="PSUM"))

        chunk_state = {}

        def ksum_tree(eng, t):
            for (a, b, n) in [(0, 24, 24), (0, 12, 12), (0, 6, 6), (0, 3, 3)]:
                eng.tensor_tensor(out=t[:, a:a + n, :], in0=t[:, a:a + n, :],
                                  in1=t[:, b:b + n, :], op=Alu.add)
            for b in (1, 2, 48):
                eng.tensor_tensor(out=t[:, 0, :], in0=t[:, 0, :],
                                  in1=t[:, b, :], op=Alu.add)


        def phase1(ci):
            (r0c, nr) = CHUNKS[ci]
            nqc = nr * W
            vc = K2 * nqc
            feat = featp.tile([66, K2, nqc], f16, tag="feat", name=f"feat{ci}")
            nc.sync.dma_start(out=feat[64:66, :, :],
                                in_=rsubp_d[:, :, r0c * W:r0c * W + nqc])

            fv = feat[0:64]
            x1v = x1s[:, r0c:r0c + nr, :]
            for di in range(K):
                for par in range(2):
                    ndj = 4 if par == 0 else 3
                    src = x2p if par == 0 else x2sh
                    x2w = win_ap(src[:], (r0c + di) * WP, ndj, nr)
                    x1w = bass.AP(tensor=x1v.tensor, offset=x1v.offset,
                                  ap=[x1v.ap[0], [0, ndj], x1v.ap[1], x1v.ap[2]])
                    outw = kq_ap(fv, di * K + par, ndj, nqc, nr)
                    eng = nc.vector if par == 0 else nc.gpsimd
                    eng.tensor_tensor(out=outw, in0=x1w, in1=x2w,
                                      op=Alu.subtract)
            for bq in range(4):
                ks = (K2 * bq) // 4, (K2 * (bq + 1)) // 4
                nc.vector.tensor_scalar_max(
                    out=feat[0:64, ks[0]:ks[1], :].rearrange("p a b -> p (a b)"),
                    in0=feat[0:64, ks[0]:ks[1], :].rearrange("p a b -> p (a b)"),
                    scalar1=0.0)

            featf = feat[:].rearrange("p a b -> p (a b)")
            h2 = h2p.tile([64, vc], f16, tag="h2")
            for j0 in range(0, vc, 1024):
                n = min(1024, vc - j0)
                ps1 = ps1p.tile([64, 1024], f32, tag="ps1")
                for s in range(0, n, 512):
                    sn = min(512, n - s)
                    nc.tensor.matmul(ps1[:, s:s + sn], cw1s[:],
                                     featf[:, j0 + s:j0 + s + sn],
                                     start=True, stop=True)
                nc.scalar.activation(out=h2[:, j0:j0 + n], in_=ps1[:, :n],
                                     func=Act.Relu, bias=b2f, scale=1.0)

            e4 = e4p.tile([128, K2, nqc], f16, tag="e4")
            e4f = e4[:].rearrange("p a b -> p (a b)")
            for j0 in range(0, vc, 1024):
                n = min(1024, vc - j0)
                ps2 = ps2p.tile([128, 1024], f32, tag="ps2")
                for s in range(0, n, 512):
                    sn = min(512, n - s)
                    nc.tensor.matmul(ps2[:, s:s + sn], cw2s[:],
                                     h2[:, j0 + s:j0 + s + sn],
                                     start=True, stop=True)
                nc.scalar.activation(out=e4f[:, j0:j0 + n], in_=ps2[:, :n],
                                     func=Act.Exp, bias=cb2r, scale=1.0)

            chunk_state[ci] = (e4,)

        def phase2(ci):
            (r0c, nr) = CHUNKS[ci]
            nqc = nr * W
            (e4,) = chunk_state[ci]
            prods = []
            for ot in range(2):
                prodt = prodp.tile([128, K2, nqc], f16, tag=f"prod{ot}",
                                   name=f"prod{ot}")
                prods.append(prodt)
                for di in range(K):
                    for par in range(2):
                        ndj = 4 if par == 0 else 3
                        srct = x3ps if par == 0 else x3shs
                        sv = srct[:, ot, :]
                        k0 = di * K + par
                        x3w = bass.AP(
                            tensor=sv.tensor,
                            offset=sv.offset + (r0c + di) * WP,
                            ap=[sv.ap[0], [2, ndj], [WP, nr], [1, W]])
                        e4w = kq_ap(e4[:], k0, ndj, nqc, nr)
                        outw = kq_ap(prods[ot][:], k0, ndj, nqc, nr)
                        nc.vector.tensor_tensor(out=outw, in0=e4w, in1=x3w,
                                                op=Alu.mult)
            ksum_tree(nc.vector, prods[0])
            ksum_tree(nc.vector, prods[1])
            zsc = zscp.tile([128, 24, nqc], f16, tag="zsc", name=f"zsc{ci}")
            nc.gpsimd.tensor_tensor(out=zsc[:, :, :], in0=e4[:, 0:24, :],
                                    in1=e4[:, 24:48, :], op=Alu.add)
            for (a, b, n) in [(0, 12, 12), (0, 6, 6), (0, 3, 3)]:
                nc.gpsimd.tensor_tensor(out=zsc[:, a:a + n, :],
                                        in0=zsc[:, a:a + n, :],
                                        in1=zsc[:, b:b + n, :], op=Alu.add)
            for bb in (1, 2):
                nc.gpsimd.tensor_tensor(out=zsc[:, 0, :], in0=zsc[:, 0, :],
                                        in1=zsc[:, bb, :], op=Alu.add)
            nc.gpsimd.tensor_tensor(out=zsc[:, 0, :], in0=zsc[:, 0, :],
                                    in1=e4[:, 48, :], op=Alu.add)

            zf = smallp.tile([128, nqc], f32, tag="zf")
            rz = smallp.tile([128, nqc], f32, tag="rz")
            nc.vector.tensor_copy(out=zf[:], in_=zsc[:, 0, :])
            nc.vector.reciprocal(out=rz[:], in_=zf[:])

            outb = []
            for ot in range(2):
                ob = smallp.tile([128, nqc], f32, tag=f"ob{ot}", name=f"ob{ot}")
                ob2 = smallp.tile([128, nqc], bf16, tag=f"ob2{ot}", name=f"ob2{ot}")
                outb.append(ob2)
                nc.vector.scalar_tensor_tensor(
                    out=ob[:], in0=prods[ot][:, 0, :], scalar=1.0, in1=rz[:],
                    op0=Alu.mult, op1=Alu.mult)
                nc.scalar.activation(out=ob2[:], in_=ob[:], func=Act.Relu,
                                     bias=b3fp[ot], scale=a3p[ot])

            for oo in range(2):
                psw = ps2p.tile([128, 1024], f32, tag="ps2")
                for kt in range(2):
                    nc.tensor.matmul(psw[:, :nqc], wcs[:, kt, oo, :],
                                     outb[kt][:],
                                     start=(kt == 0), stop=(kt == 1))
                ysb = smallp.tile([128, nqc], f32, tag=f"ysb{oo}", name=f"ysb{oo}")
                xi = xps[oo][:, PAD + r0c:PAD + r0c + nr, PAD:PAD + W]
                nc.vector.scalar_tensor_tensor(
                    out=ysb[:], in0=psw[:, :nqc], scalar=bcb[oo], in1=xi,
                    op0=Alu.add, op1=Alu.add)
                nc.sync.dma_start(
                    out=y_d[oo][:, r0c:r0c + nr, :],
                    in_=ysb[:].rearrange("p (r w) -> p r w", w=W))

        phase1(0)
        for ci in range(1, len(CHUNKS)):
            phase1(ci)
            phase2(ci - 1)
        phase2(len(CHUNKS) - 1)

    nc.compile()
    _BUILD_CACHE["nc"] = nc
    return nc


def _host_prep(inputs):
    f = {k: np.asarray(v, np.float32) for k, v in inputs.items()}

    def fold(n):
        a = f[n + "_g"] / np.sqrt(f[n + "_rv"] + EPS)
        return a, f[n + "_b"] - f[n + "_rm"] * a

    a1, b1f = fold("bn1")
    ac, bc1 = fold("cwbn1")
    a2, b2f = fold("cwbn2")
    a3, b3f = fold("bn2")

    W1p = ac[:64, None] * f["w1"]
    b1p = ac[:64] * f["b1"] + bc1[:64]
    W2p = ac[:64, None] * f["w2"]
    b2p = ac[:64] * f["b2"]
    cw1p = a2[:, None] * f["cw1"]

    perm = _perm_channels()
    w3p = f["w3"][perm]
    a3p = a3[perm]
    b3fp = b3f[perm]
    rep = np.arange(128) // 4
    cw2r = f["cw2"][rep]
    cb2r = f["cb2"][rep]

    locw = np.tile(np.linspace(-1.0, 1.0, W, dtype=np.float32)[None, :], (H, 1))
    loch = np.tile(np.linspace(-1.0, 1.0, H, dtype=np.float32)[:, None], (1, W))
    loc = np.stack([locw, loch], 0)
    p = np.einsum("chw,oc->ohw", loc, f["pw"]) + f["pb"][:, None, None]
    pp = np.pad(p, ((0, 0), (PAD, PAD), (PAD, PAD)), mode="reflect")
    pu = np.stack([pp[:, i:i + H, j:j + W] for i in range(K) for j in range(K)], 1)
    subp = p[:, None] - pu
    rsubp = np.maximum(ac[64:66, None, None, None] * subp
                       + bc1[64:66, None, None, None], 0).astype(np.float16)

    xpad = np.pad(f["x"], ((0, 0), (0, 0), (PAD, PAD), (PAD, PAD)), mode="reflect")

    w1T = np.ascontiguousarray(W1p.T).reshape(2, 128, 64).copy()
    w2T = np.ascontiguousarray(W2p.T).reshape(2, 128, 64).copy()
    w3T = np.empty((2, 128, 2, 128), np.float32)
    wcT = np.empty((2, 128, 2, 128), np.float32)
    wc_perm = f["wc"][:, perm]
    for kt in range(2):
        for ot in range(2):
            w3T[kt, :, ot, :] = w3p[ot * 128:(ot + 1) * 128,
                                    kt * 128:(kt + 1) * 128].T
            wcT[kt, :, ot, :] = wc_perm[ot * 128:(ot + 1) * 128,
                                        kt * 128:(kt + 1) * 128].T
    cw1T = np.ascontiguousarray(cw1p.T).astype(np.float16)
    cw2T = np.ascontiguousarray(cw2r.T).astype(np.float16)

    scal = np.zeros((128, 14), np.float32)
    scal[:, 0] = a1[:128]; scal[:, 1] = a1[128:]
    scal[:, 2] = b1f[:128]; scal[:, 3] = b1f[128:]
    scal[:64, 4] = b1p; scal[:64, 5] = b2p; scal[:64, 6] = b2f
    scal[:, 7] = cb2r
    scal[:, 8] = a3p[:128]; scal[:, 9] = a3p[128:]
    scal[:, 10] = b3fp[:128]; scal[:, 11] = b3fp[128:]
    scal[:, 12] = f["bc"][:128]; scal[:, 13] = f["bc"][128:]

    shared = dict(w1T=w1T.astype(bf16_np), w2T=w2T.astype(bf16_np),
                  w3T=w3T.astype(bf16_np), wcT=wcT.astype(bf16_np),
                  cw1T=cw1T, cw2T=cw2T, scal=scal)
    in_maps = []
    for core in range(8):
        b, i = divmod(core, 4)
        r0 = RB * i
        m = dict(shared)
        m["xp"] = np.ascontiguousarray(
            xpad[b].reshape(2, 128, H + 2 * PAD, WP)[:, :, r0:r0 + ROWS, :])
        m["rsubp"] = np.ascontiguousarray(
            rsubp[:, :, r0:r0 + RB, :].reshape(2, K2, NQ))
        in_maps.append(m)
    return in_maps


def kernel(**inputs):
    from concourse.bass_utils import run_bass_kernel_spmd
    nc = _build_program()
    in_maps = _host_prep(inputs)
    res = run_bass_kernel_spmd(nc, in_maps, core_ids=list(range(8)))
    global LAST_RESULTS
    LAST_RESULTS = res
    y = np.zeros((B, C, H, W), np.float32)
    for core in range(8):
        b, i = divmod(core, 4)
        yc = res.results[core]["y"]
        y[b, :, RB * i:RB * (i + 1), :] = yc.reshape(C, RB, W)
    return y



# revision 30
# speedup vs baseline: 1.2854x; 1.2854x over previous
"""SAN Bottleneck (pairwise self-attention) Trainium2 kernel.

Sharding: 8 cores = 2 batches x 4 row-blocks of 14 rows (H=56). Each core
receives a reflect-padded input slice (20 rows x 62 cols), so the 7x7
unfold needs no runtime halo exchange and no edge special-casing.

Per-core pipeline (batchnorms folded into per-channel scale/bias on host):
  bn1+relu -> x1/x2/x3 1x1 convs (bf16 matmuls).
  Query rows are PARITY-PACKED: partitions 0:64 = feature channels for
  even query rows, 64:128 = odd rows. x1q/x2q are produced directly in
  that layout (x2q's odd block is the same conv shifted one padded row),
  so feat = relu(x1 - win(x2)) runs at full 128-partition width and the
  conv_w matmul chain does half the per-partition work.
  mm1 is one 128-contraction block-diagonal matmul; the position branch
  accumulates into the same PSUM via a 4-partition matmul.
  mm2 (64->128, head weights replicated rep-major: partition p = head
  p%32) produces e4 = exp(logits) for both parities in one PSUM evac.
  Aggregation: per-tap window products vs x3 (s-split rep-major channels)
  + in-place pairwise tree sums, split across DVE and GPSIMD (GPSIMD ops
  issued as scalar_tensor_tensor).
  Softmax normalizer: e4's head rows are DMA-packed 4 query-quarters wide
  [128 = 4q x 32h], tree-summed at 1/4 cost, DMA-unpacked to [32, nq],
  replicated across partition blocks by a 0/1 matmul, reciprocal, divide.
  bn2+relu -> wc conv + bias + identity residual.
"""

import numpy as np
import ml_dtypes

bf16_np = ml_dtypes.bfloat16

K = 7
PAD = 3
EPS = 1e-5
B, C, H, W = 2, 256, 56, 56
RB = 14              # rows per core
ROWS = RB + 2 * PAD  # 20
WP = W + 2 * PAD     # 62
K2 = K * K
NP2 = RB // 2        # 7 row-pairs
CHUNKS = [(0, 2), (2, 2), (4, 2), (6, 1)]   # (row-pair offset, row-pairs)

_BUILD_CACHE = {}


def _perm_channels():
    # rep-major s-split: partition p of ot tile t <-> channel 8*(p%32)+4t+(p//32)
    perm = np.zeros(256, np.int64)
    for t in range(2):
        for p in range(128):
            perm[t * 128 + p] = 8 * (p % 32) + 4 * t + (p // 32)
    return perm


def _build_program():
    if "nc" in _BUILD_CACHE:
        return _BUILD_CACHE["nc"]
    import concourse.bass as bass
    import concourse.bacc as bacc
    import concourse.tile as tile
    import concourse.mybir as mybir
    from contextlib import ExitStack

    f32 = mybir.dt.float32
    f16 = mybir.dt.float16
    bf16 = mybir.dt.bfloat16
    Alu = mybir.AluOpType
    Act = mybir.ActivationFunctionType

    nc = bacc.Bacc("TRN2", target_bir_lowering=False, num_devices=8)

    xp_d = nc.dram_tensor("xp", [2, 128, ROWS, WP], f32, kind="ExternalInput")
    rsubp_d = nc.dram_tensor("rsubp", [4, K2, NP2, W], f16, kind="ExternalInput")
    wb_d = nc.dram_tensor("wb", [128, 1280], bf16, kind="ExternalInput")
    fb_d = nc.dram_tensor("fb", [128, 512], f16, kind="ExternalInput")
    scal_d = nc.dram_tensor("scal", [128, 14], f32, kind="ExternalInput")
    y_d = nc.dram_tensor("y", [2, 128, RB, W], f32, kind="ExternalOutput")

    with tile.TileContext(nc) as tc, ExitStack() as stack:
        consts = stack.enter_context(tc.tile_pool(name="consts", bufs=1))
        xpp = stack.enter_context(tc.tile_pool(name="xpp", bufs=1))
        headsb = stack.enter_context(tc.tile_pool(name="headsb", bufs=1))

        wb = consts.tile([128, 1280], bf16, tag="wb")
        fb = consts.tile([128, 512], f16, tag="fb")
        scals = consts.tile([128, 14], f32, tag="scals")
        nc.sync.dma_start(out=wb[:], in_=wb_d[:])
        nc.sync.dma_start(out=fb[:], in_=fb_d[:])
        nc.sync.dma_start(out=scals[:], in_=scal_d[:])
        w1s = wb[:, 0:128].rearrange("p (k a) -> p k a", k=2)
        w2s = wb[:, 128:256].rearrange("p (k a) -> p k a", k=2)
        w3s = wb[:, 256:768].rearrange("p (k o a) -> p k o a", k=2, o=2)
        wcs = wb[:, 768:1280].rearrange("p (k o a) -> p k o a", k=2, o=2)
        cw1s = fb[:, 0:128]
        cw2s = fb[:, 128:256]
        posws = fb[0:4, 256:384]
        idents = fb[:, 384:512]

        a1 = [scals[:, 0:1], scals[:, 1:2]]
        b1f = [scals[:, 2:3], scals[:, 3:4]]
        b1p = scals[:, 4:5]
        b2p = scals[:, 5:6]
        b2f = scals[:, 6:7]
        cb2r = scals[:, 7:8]
        a3p = [scals[:, 8:9], scals[:, 9:10]]
        b3fp = [scals[:, 10:11], scals[:, 11:12]]
        bcb = [scals[:, 12:13], scals[:, 13:14]]

        xps = [xpp.tile([128, ROWS, WP], f32, tag=f"xp{t}", name=f"xp{t}")
               for t in range(2)]
        for t in range(2):
            nc.sync.dma_start(out=xps[t][:], in_=xp_d[t])
        obn = [headsb.tile([128, ROWS * WP], bf16, tag=f"obn{t}", name=f"obn{t}")
               for t in range(2)]
        for t in range(2):
            nc.scalar.activation(
                out=obn[t][:],
                in_=xps[t][:].rearrange("p r w -> p (r w)"),
                func=Act.Relu, bias=b1f[t], scale=a1[t])

        x1q = headsb.tile([128, NP2, W], f16, tag="x1q")
        x2q = headsb.tile([128, ROWS * WP], f16, tag="x2q")
        x3q = headsb.tile([128, 2, ROWS * WP], f16, tag="x3q")

        ccuts = [(0, 512), (512, 512), (1024, 216)]
        with tc.tile_pool(name="pshead", bufs=1, space="PSUM") as pshead:
            # x2q: partition block b = x2 over padded rows shifted by b.
            ps = pshead.tile([128, 1240], f32, tag="psx2")
            for (o0, n) in ccuts:
                mms = []
                for b in range(2):
                    lo = b * WP
                    nn_ = max(0, min(n, ROWS * WP - lo - o0))
                    if nn_ == 0:
                        continue
                    for kt in range(2):
                        mms.append((b, kt, nn_, lo))
                for (b, kt, nn_, lo) in mms:
                    nc.tensor.matmul(
                        ps[64 * b:64 * b + 64, o0:o0 + nn_],
                        w2s[:, kt, :],
                        obn[kt][:, lo + o0:lo + o0 + nn_],
                        start=(kt == 0), stop=(kt == 1),
                        skip_group_check=(b == 1))
            nc.scalar.activation(out=x2q[:, 0:1178], in_=ps[:, 0:1178],
                                 func=Act.Identity, bias=b2p, scale=1.0)
            nc.scalar.activation(out=x2q[0:64, 1178:1240],
                                 in_=ps[0:64, 1178:1240],
                                 func=Act.Identity, bias=scals[0:64, 5:6],
                                 scale=1.0)

            # x1q: partition block b = x1 at query rows 2t+b.
            ps1 = pshead.tile([128, NP2 * W], f32, tag="psx1")
            for b in range(2):
                for kt in range(2):
                    ob = obn[kt][:].rearrange("p (r w) -> p r w", w=WP)
                    rhs = bass.AP(
                        tensor=ob.tensor,
                        offset=ob.offset + (PAD + b) * WP + PAD,
                        ap=[ob.ap[0], [2 * WP, NP2], [1, W]])
                    nc.tensor.matmul(
                        ps1[64 * b:64 * b + 64, :],
                        w1s[:, kt, :], rhs,
                        start=(kt == 0), stop=(kt == 1),
                        skip_group_check=(b == 1))
            nc.scalar.activation(
                out=x1q[:].rearrange("p a b -> p (a b)"), in_=ps1[:],
                func=Act.Identity, bias=b1p, scale=1.0)

            # x3q: s-split rep-major channels over the full padded region.
            for ot in range(2):
                ps3 = pshead.tile([128, 1240], f32, tag="psx3", name=f"psx3_{ot}")
                for (o0, n) in ccuts:
                    for kt in range(2):
                        nc.tensor.matmul(
                            ps3[:, o0:o0 + n], w3s[:, kt, ot, :],
                            obn[kt][:, o0:o0 + n],
                            start=(kt == 0), stop=(kt == 1))
                nc.scalar.activation(out=x3q[:, ot, :], in_=ps3[:], func=Act.Copy)

        rsubpp = stack.enter_context(tc.tile_pool(name="rsubpp", bufs=2))
        ysballp = stack.enter_context(tc.tile_pool(name="ysballp", bufs=1))
        ysball = ysballp.tile([128, 2, NP2, 2, W], f32, tag="ysball")
        featp = stack.enter_context(tc.tile_pool(name="featp", bufs=2))
        h2p = stack.enter_context(tc.tile_pool(name="h2p", bufs=1))
        e4p = stack.enter_context(tc.tile_pool(name="e4p", bufs=2))
        prodp = stack.enter_context(tc.tile_pool(name="prodp", bufs=1))
        smallp = stack.enter_context(tc.tile_pool(name="smallp", bufs=2))
        ps1p = stack.enter_context(tc.tile_pool(name="ps1p", bufs=2, space="PSUM"))
        ps2p = stack.enter_context(tc.tile_pool(name="ps2p", bufs=2, space="PSUM"))
        paggp = stack.enter_context(tc.tile_pool(name="paggp", bufs=1, space="PSUM"))

        chunk_state = {}

        MB1 = 512     # mm1 psum block cols
        MB2 = 512     # mm2 psum block cols per parity (tile is 2x this)

        def phase1(ci):
            (t0, np_) = CHUNKS[ci]
            nq2 = np_ * W
            vc2 = K2 * nq2

            rsub = rsubpp.tile([4, K2, np_, W], f16, tag="rsub", name=f"rsub{ci}")
            nc.sync.dma_start(out=rsub[:], in_=rsubp_d[:, :, t0:t0 + np_, :])

            feat = featp.tile([128, K2, nq2], f16, tag="feat", name=f"feat{ci}")
            # feat[p, di*7+dj, t, w] = x1q[p, t0+t, w] - x2q[p, 2(t0+t)+di, dj+w]
            x1v = x1q[:]
            x2v = x2q[:]
            fv = feat[:]
            for di in range(K):
                x1w = bass.AP(
                    tensor=x1v.tensor, offset=x1v.offset + t0 * W,
                    ap=[x1v.ap[0], [0, K], [W, np_], [1, W]])
                x2w = bass.AP(
                    tensor=x2v.tensor, offset=x2v.offset + (2 * t0 + di) * WP,
                    ap=[x2v.ap[0], [1, K], [2 * WP, np_], [1, W]])
                outw = bass.AP(
                    tensor=fv.tensor, offset=fv.offset + di * K * nq2,
                    ap=[fv.ap[0], [nq2, K], [W, np_], [1, W]])
                eng = nc.gpsimd if di >= 5 else nc.vector
                eng.tensor_tensor(out=outw, in0=x1w, in1=x2w,
                                  op=Alu.subtract)
            featf = feat[:].rearrange("p a b -> p (a b)")
            nc.vector.tensor_scalar_max(out=featf, in0=featf, scalar1=0.0)

            h2 = h2p.tile([128, vc2], f16, tag="h2")
            e4 = e4p.tile([128, 2, K2, nq2], f16, tag="e4", name=f"e4{ci}")
            rsf = rsub[:].rearrange("p a b c -> p (a b c)")
            del rsub
            for j0 in range(0, vc2, MB1):
                n = min(MB1, vc2 - j0)
                psa = ps1p.tile([128, MB1], f32, tag="ps1")
                for s in range(0, n, 512):
                    sn = min(512, n - s)
                    nc.tensor.matmul(psa[:, s:s + sn], cw1s,
                                     featf[:, j0 + s:j0 + s + sn],
                                     start=True, stop=False)
                    nc.tensor.matmul(psa[:, s:s + sn], posws,
                                     rsf[:, j0 + s:j0 + s + sn],
                                     start=False, stop=True)
                nc.scalar.activation(out=h2[:, j0:j0 + n], in_=psa[:, :n],
                                     func=Act.Relu, bias=b2f, scale=1.0)
            for j0 in range(0, vc2, MB2):
                n = min(MB2, vc2 - j0)
                psb = ps2p.tile([128, 2 * MB2], f32, tag="ps2")
                for par in range(2):
                    nc.tensor.matmul(
                        psb[:, par * MB2:par * MB2 + n],
                        cw2s[64 * par:64 * par + 64, :],
                        h2[64 * par:64 * par + 64, j0:j0 + n],
                        start=True, stop=True)
                e4w = bass.AP(
                    tensor=e4.tensor, offset=e4[:].offset + j0,
                    ap=[e4[:].ap[0], [K2 * nq2, 2], [1, n]])
                psbv = psb[:]
                psin = bass.AP(tensor=psb.tensor, offset=psbv.offset,
                               ap=[psbv.ap[0], [MB2, 2], [1, n]])
                nc.scalar.activation(out=e4w, in_=psin,
                                     func=Act.Exp, bias=cb2r, scale=1.0)
            chunk_state[ci] = (e4,)

        def phase2(ci):
            (t0, np_) = CHUNKS[ci]
            nq2 = np_ * W
            (e4,) = chunk_state[ci]

            # normalizer: PE identity-accumulation over taps (both parities)
            pz = paggp.tile([128, 2 * nq2], f32, tag="pz", name=f"pz{ci}")
            for k in range(K2):
                nc.tensor.matmul(
                    pz[:], idents, e4[:, :, k, :],
                    start=(k == 0), stop=(k == K2 - 1))
            rz = smallp.tile([128, 2, nq2], f16, tag="rz", name=f"rz{ci}")
            with nc.allow_low_precision(reason="fp16 softmax reciprocal"):
                nc.vector.reciprocal(
                    out=rz[:].rearrange("p a b -> p (a b)"), in_=pz[:])

            for par in range(2):
                e4v = e4[:, par, :, :]
                prodt = prodp.tile([128, 2, K2, nq2], f16, tag=f"prod{par}",
                                   name=f"prod{par}")
                pv = prodt[:]
                for di in range(K):
                    for ot in range(2):
                        e4w = bass.AP(
                            tensor=e4.tensor,
                            offset=e4v.offset + di * K * nq2,
                            ap=[e4v.ap[0], [nq2, K], [W, np_], [1, W]])
                        x3w = bass.AP(
                            tensor=x3q.tensor,
                            offset=x3q[:].offset + ot * ROWS * WP
                            + (2 * t0 + par + di) * WP,
                            ap=[x3q[:].ap[0], [1, K], [2 * WP, np_], [1, W]])
                        outw = bass.AP(
                            tensor=prodt.tensor,
                            offset=pv.offset + ot * K2 * nq2 + di * K * nq2,
                            ap=[pv.ap[0], [nq2, K], [W, np_], [1, W]])
                        eng = nc.gpsimd if di >= 5 else nc.vector
                        eng.tensor_tensor(out=outw, in0=e4w, in1=x3w,
                                          op=Alu.mult)
                # numerator: PE identity-accumulation over taps (both ot)
                pnum = paggp.tile([128, 2 * nq2], f32, tag="pnum",
                                  name=f"pnum{par}")
                for k in range(K2):
                    nc.tensor.matmul(
                        pnum[:], idents, pv[:, :, k, :],
                        start=(k == 0), stop=(k == K2 - 1))

                # divide + bn2/relu + wc + residual
                ob = smallp.tile([128, 2, nq2], f32, tag="ob", name=f"ob{par}")
                rzw = bass.AP(
                    tensor=rz.tensor, offset=rz[:].offset + par * nq2,
                    ap=[rz[:].ap[0], [0, 2], [1, nq2]])
                nc.vector.tensor_tensor(
                    out=ob[:], in0=pnum[:].rearrange("p (a b) -> p a b", a=2),
                    in1=rzw, op=Alu.mult)
                ob2 = smallp.tile([128, 2, nq2], bf16, tag="ob2",
                                  name=f"ob2{par}")
                for ot in range(2):
                    nc.scalar.activation(out=ob2[:, ot, :], in_=ob[:, ot, :],
                                         func=Act.Relu, bias=b3fp[ot],
                                         scale=a3p[ot])
                for oo in range(2):
                    psw = paggp.tile([128, 224], f32, tag="pz",
                                    name=f"pswc{par}{oo}")
                    for kt in range(2):
                        nc.tensor.matmul(psw[:, :nq2], wcs[:, kt, oo, :],
                                         ob2[:, kt, :],
                                         start=(kt == 0), stop=(kt == 1))
                    xi = bass.AP(
                        tensor=xps[oo].tensor,
                        offset=xps[oo][:].offset
                        + (PAD + 2 * t0 + par) * WP + PAD,
                        ap=[xps[oo][:].ap[0], [2 * WP, np_], [1, W]])
                    nc.vector.scalar_tensor_tensor(
                        out=ysball[:, oo, t0:t0 + np_, par, :],
                        in0=psw[:, :nq2].rearrange("p (r w) -> p r w", w=W),
                        scalar=bcb[oo], in1=xi,
                        op0=Alu.add, op1=Alu.add)

        phase1(0)
        for ci in range(1, len(CHUNKS)):
            phase1(ci)
            phase2(ci - 1)
        phase2(len(CHUNKS) - 1)
        for oo in range(2):
            nc.sync.dma_start(
                out=y_d[oo],
                in_=ysball[:, oo, :, :, :].rearrange(
                    "p t q w -> p (t q) w"))

    nc.compile()
    _BUILD_CACHE["nc"] = nc
    return nc


def _host_prep(inputs):
    f = {k: np.asarray(v, np.float32) for k, v in inputs.items()}

    def fold(n):
        a = f[n + "_g"] / np.sqrt(f[n + "_rv"] + EPS)
        return a, f[n + "_b"] - f[n + "_rm"] * a

    a1, b1f = fold("bn1")
    ac, bc1 = fold("cwbn1")
    a2, b2f = fold("cwbn2")
    a3, b3f = fold("bn2")

    W1p = ac[:64, None] * f["w1"]
    b1p = ac[:64] * f["b1"] + bc1[:64]
    W2p = ac[:64, None] * f["w2"]
    b2p = ac[:64] * f["b2"]
    cw1p = a2[:, None] * f["cw1"]          # [64, 66]

    perm = _perm_channels()
    w3p = f["w3"][perm]
    a3p = a3[perm]
    b3fp = b3f[perm]
    rep = np.arange(128) % 32
    cw2r = f["cw2"][rep]
    cb2r = f["cb2"][rep]

    locw = np.tile(np.linspace(-1.0, 1.0, W, dtype=np.float32)[None, :], (H, 1))
    loch = np.tile(np.linspace(-1.0, 1.0, H, dtype=np.float32)[:, None], (1, W))
    loc = np.stack([locw, loch], 0)
    p = np.einsum("chw,oc->ohw", loc, f["pw"]) + f["pb"][:, None, None]
    pp = np.pad(p, ((0, 0), (PAD, PAD), (PAD, PAD)), mode="reflect")
    pu = np.stack([pp[:, i:i + H, j:j + W] for i in range(K) for j in range(K)], 1)
    subp = p[:, None] - pu
    rsubp = np.maximum(ac[64:66, None, None, None] * subp
                       + bc1[64:66, None, None, None], 0).astype(np.float16)

    xpad = np.pad(f["x"], ((0, 0), (0, 0), (PAD, PAD), (PAD, PAD)), mode="reflect")

    w1T = np.ascontiguousarray(W1p.T).reshape(2, 128, 64).copy()
    w2T = np.ascontiguousarray(W2p.T).reshape(2, 128, 64).copy()
    w3T = np.empty((2, 128, 2, 128), np.float32)
    wcT = np.empty((2, 128, 2, 128), np.float32)
    wc_perm = f["wc"][:, perm]
    for kt in range(2):
        for ot in range(2):
            w3T[kt, :, ot, :] = w3p[ot * 128:(ot + 1) * 128,
                                    kt * 128:(kt + 1) * 128].T
            wcT[kt, :, ot, :] = wc_perm[ot * 128:(ot + 1) * 128,
                                        kt * 128:(kt + 1) * 128].T

    cw1T = np.zeros((128, 128), np.float32)
    cw1T[0:64, 0:64] = cw1p[:, 0:64].T
    cw1T[64:128, 64:128] = cw1p[:, 0:64].T
    poswT = np.zeros((4, 128), np.float32)
    poswT[0:2, 0:64] = cw1p[:, 64:66].T
    poswT[2:4, 64:128] = cw1p[:, 64:66].T
    cw2T = np.zeros((128, 128), np.float32)
    cw2T[0:64, :] = cw2r.T
    cw2T[64:128, :] = cw2r.T


    scal = np.zeros((128, 14), np.float32)
    scal[:, 0] = a1[:128]; scal[:, 1] = a1[128:]
    scal[:, 2] = b1f[:128]; scal[:, 3] = b1f[128:]
    scal[:64, 4] = b1p; scal[64:, 4] = b1p
    scal[:64, 5] = b2p; scal[64:, 5] = b2p
    scal[:64, 6] = b2f; scal[64:, 6] = b2f
    scal[:, 7] = cb2r
    scal[:, 8] = a3p[:128]; scal[:, 9] = a3p[128:]
    scal[:, 10] = b3fp[:128]; scal[:, 11] = b3fp[128:]
    scal[:, 12] = f["bc"][:128]; scal[:, 13] = f["bc"][128:]

    wb = np.zeros((128, 1280), np.float32)
    wb[:, 0:64] = w1T[0]; wb[:, 64:128] = w1T[1]
    wb[:, 128:192] = w2T[0]; wb[:, 192:256] = w2T[1]
    wb[:, 256:768] = w3T.transpose(1, 0, 2, 3).reshape(128, 512)
    wb[:, 768:1280] = wcT.transpose(1, 0, 2, 3).reshape(128, 512)
    fbb = np.zeros((128, 512), np.float32)
    fbb[:, 0:128] = cw1T
    fbb[:, 128:256] = cw2T
    fbb[0:4, 256:384] = poswT
    fbb[:, 384:512] = np.eye(128, dtype=np.float32)
    shared = dict(wb=wb.astype(bf16_np), fb=fbb.astype(np.float16),
                  scal=scal)
    in_maps = []
    for core in range(8):
        b, i = divmod(core, 4)
        r0 = RB * i
        m = dict(shared)
        m["xp"] = np.ascontiguousarray(
            xpad[b].reshape(2, 128, H + 2 * PAD, WP)[:, :, r0:r0 + ROWS, :])
        rs = rsubp[:, :, r0:r0 + RB, :]           # [2, 49, 14, 56]
        rs4 = np.empty((4, K2, NP2, W), np.float16)
        rs4[0:2] = rs[:, :, 0::2, :]
        rs4[2:4] = rs[:, :, 1::2, :]
        m["rsubp"] = np.ascontiguousarray(rs4)
        in_maps.append(m)
    return in_maps


def kernel(**inputs):
    from concourse.bass_utils import run_bass_kernel_spmd
    nc = _build_program()
    in_maps = _host_prep(inputs)
    res = run_bass_kernel_spmd(nc, in_maps, core_ids=list(range(8)))
    global LAST_RESULTS
    LAST_RESULTS = res
    y = np.zeros((B, C, H, W), np.float32)
    for core in range(8):
        b, i = divmod(core, 4)
        yc = res.results[core]["y"]
        y[b, :, RB * i:RB * (i + 1), :] = yc.reshape(C, RB, W)
    return y


# revision 45
# speedup vs baseline: 1.4553x; 1.1322x over previous
"""SAN Bottleneck (pairwise self-attention) Trainium2 kernel.

Sharding: 8 cores = 2 batches x 4 row-blocks of 14 rows (H=56). Each core
receives a reflect-padded input slice (20 rows x 62 cols), so the 7x7
unfold needs no runtime halo exchange and no edge special-casing.

Per-core pipeline (batchnorms folded into per-channel scale/bias on host):
  bn1+relu -> x1/x2/x3 1x1 convs (bf16 matmuls).
  Query rows are PARITY-PACKED: partitions 0:64 = feature channels for
  even query rows, 64:128 = odd rows. x1q/x2q are produced directly in
  that layout (x2q's odd block is the same conv shifted one padded row),
  so feat = relu(x1 - win(x2)) runs at full 128-partition width and the
  conv_w matmul chain does half the per-partition work.
  mm1 is one 128-contraction block-diagonal matmul; the position branch
  accumulates into the same PSUM via a 4-partition matmul.
  mm2 (64->128, head weights replicated rep-major: partition p = head
  p%32) produces e4 = exp(logits) for both parities in one PSUM evac.
  Aggregation: per-tap window products vs x3 (s-split rep-major channels)
  + in-place pairwise tree sums, split across DVE and GPSIMD (GPSIMD ops
  issued as scalar_tensor_tensor).
  Softmax normalizer: e4's head rows are DMA-packed 4 query-quarters wide
  [128 = 4q x 32h], tree-summed at 1/4 cost, DMA-unpacked to [32, nq],
  replicated across partition blocks by a 0/1 matmul, reciprocal, divide.
  bn2+relu -> wc conv + bias + identity residual.
"""

import numpy as np
import ml_dtypes

bf16_np = ml_dtypes.bfloat16

K = 7
PAD = 3
EPS = 1e-5
B, C, H, W = 2, 256, 56, 56
RB = 14              # rows per core
ROWS = RB + 2 * PAD  # 20
WP = W + 2 * PAD     # 62
K2 = K * K
NP2 = RB // 2        # 7 row-pairs
CHUNKS = [(0, 2), (2, 2), (4, 2), (6, 1)]   # (row-pair offset, row-pairs)

_BUILD_CACHE = {}


def _perm_channels():
    # rep-major s-split: partition p of ot tile t <-> channel 8*(p%32)+4t+(p//32)
    perm = np.zeros(256, np.int64)
    for t in range(2):
        for p in range(128):
            perm[t * 128 + p] = 8 * (p % 32) + 4 * t + (p // 32)
    return perm


def _build_program():
    if "nc" in _BUILD_CACHE:
        return _BUILD_CACHE["nc"]
    import concourse.bass as bass
    import concourse.bacc as bacc
    import concourse.tile as tile
    import concourse.mybir as mybir
    from contextlib import ExitStack

    f32 = mybir.dt.float32
    f16 = mybir.dt.float16
    bf16 = mybir.dt.bfloat16
    Alu = mybir.AluOpType
    Act = mybir.ActivationFunctionType

    nc = bacc.Bacc("TRN2", target_bir_lowering=False, num_devices=8)

    xp_d = nc.dram_tensor("xp", [2, 128, ROWS, WP], f32, kind="ExternalInput")
    rsubp_d = nc.dram_tensor("rsubp", [4, K2, NP2, W], f16, kind="ExternalInput")
    wb_d = nc.dram_tensor("wb", [128, 1280], bf16, kind="ExternalInput")
    fb_d = nc.dram_tensor("fb", [128, 640], f16, kind="ExternalInput")
    scal_d = nc.dram_tensor("scal", [128, 14], f32, kind="ExternalInput")
    y_d = nc.dram_tensor("y", [2, 128, RB, W], f32, kind="ExternalOutput")

    with tile.TileContext(nc) as tc, ExitStack() as stack:
        consts = stack.enter_context(tc.tile_pool(name="consts", bufs=1))
        xpp = stack.enter_context(tc.tile_pool(name="xpp", bufs=1))
        headsb = stack.enter_context(tc.tile_pool(name="headsb", bufs=1))

        wb = consts.tile([128, 1280], bf16, tag="wb")
        fb = consts.tile([128, 640], f16, tag="fb")
        scals = consts.tile([128, 14], f32, tag="scals")
        nc.sync.dma_start(out=scals[:], in_=scal_d[:])
        w1s = wb[:, 0:128].rearrange("p (k a) -> p k a", k=2)
        w2s = wb[:, 128:256].rearrange("p (k a) -> p k a", k=2)
        w3s = wb[:, 256:768].rearrange("p (k o a) -> p k o a", k=2, o=2)
        wcs = wb[:, 768:1280].rearrange("p (k o a) -> p k o a", k=2, o=2)
        cw1s = fb[:, 0:128]
        cw2s = fb[:, 128:256]
        posws = fb[0:4, 256:384]
        idents = fb[:, 384:512]
        repms = fb[0:32, 512:640]

        a1 = [scals[:, 0:1], scals[:, 1:2]]
        b1f = [scals[:, 2:3], scals[:, 3:4]]
        b1p = scals[:, 4:5]
        b2p = scals[:, 5:6]
        b2f = scals[:, 6:7]
        cb2r = scals[:, 7:8]
        a3p = [scals[:, 8:9], scals[:, 9:10]]
        b3fp = [scals[:, 10:11], scals[:, 11:12]]
        bcb = [scals[:, 12:13], scals[:, 13:14]]

        xps = [xpp.tile([128, ROWS, WP], f32, tag=f"xp{t}", name=f"xp{t}")
               for t in range(2)]
        nc.sync.dma_start(out=xps[0][:], in_=xp_d[0])
        nc.sync.dma_start(out=wb[:], in_=wb_d[:])
        nc.sync.dma_start(out=xps[1][:], in_=xp_d[1])
        nc.sync.dma_start(out=fb[:], in_=fb_d[:])
        # PE warmup: keep the PE busy during input load so the clock gate
        # opens before the head convolutions.
        warm = consts.tile([128, 16], f16, tag="warm")
        nc.vector.memset(warm[:], 0.0)
        warma = consts.tile([128, 16], f16, tag="warma")
        nc.scalar.activation(out=warma[:], in_=warm[:], func=Act.Exp,
                             bias=0.0, scale=1.0)
        obn = [headsb.tile([128, ROWS * WP], bf16, tag=f"obn{t}", name=f"obn{t}")
               for t in range(2)]
        nc.scalar.activation(
            out=obn[0][:],
            in_=xps[0][:].rearrange("p r w -> p (r w)"),
            func=Act.Relu, bias=b1f[0], scale=a1[0])
        nc.vector.tensor_scalar(
            out=obn[1][:], in0=xps[1][:].rearrange("p r w -> p (r w)"),
            scalar1=a1[1], scalar2=b1f[1], op0=Alu.mult, op1=Alu.add)
        nc.vector.tensor_scalar_max(out=obn[1][:], in0=obn[1][:], scalar1=0.0)

        x1q = headsb.tile([128, NP2, W], f16, tag="x1q")
        x2q = headsb.tile([128, ROWS * WP], f16, tag="x2q")
        x3q = headsb.tile([128, 2, ROWS * WP], f16, tag="x3q")

        ccuts = [(0, 512), (512, 512), (1024, 216)]
        with tc.tile_pool(name="pshead", bufs=1, space="PSUM") as pshead:
            pwarm = pshead.tile([128, 512], f32, tag="pwarm")
            wrhs = bass.AP(tensor=warm.tensor, offset=warm[:].offset,
                           ap=[warm[:].ap[0], [0, 32], [1, 16]])
            for wi in range(8):
                nc.tensor.matmul(pwarm[0:16, :], warm[:], wrhs,
                                 start=True, stop=True)
            # x2q: partition block b = x2 over padded rows shifted by b.
            ps = pshead.tile([128, 1240], f32, tag="psx2")
            for (o0, n) in ccuts:
                mms = []
                for b in range(2):
                    lo = b * WP
                    nn_ = max(0, min(n, ROWS * WP - lo - o0))
                    if nn_ == 0:
                        continue
                    for kt in range(2):
                        mms.append((b, kt, nn_, lo))
                for (b, kt, nn_, lo) in mms:
                    nc.tensor.matmul(
                        ps[64 * b:64 * b + 64, o0:o0 + nn_],
                        w2s[:, kt, :],
                        obn[kt][:, lo + o0:lo + o0 + nn_],
                        start=(kt == 0), stop=(kt == 1),
                        skip_group_check=(b == 1))
            nc.scalar.activation(out=x2q[:, 0:1178], in_=ps[:, 0:1178],
                                 func=Act.Identity, bias=b2p, scale=1.0)
            nc.scalar.activation(out=x2q[0:64, 1178:1240],
                                 in_=ps[0:64, 1178:1240],
                                 func=Act.Identity, bias=scals[0:64, 5:6],
                                 scale=1.0)

            # x1q: partition block b = x1 at query rows 2t+b.
            ps1 = pshead.tile([128, NP2 * W], f32, tag="psx1")
            for b in range(2):
                for kt in range(2):
                    ob = obn[kt][:].rearrange("p (r w) -> p r w", w=WP)
                    rhs = bass.AP(
                        tensor=ob.tensor,
                        offset=ob.offset + (PAD + b) * WP + PAD,
                        ap=[ob.ap[0], [2 * WP, NP2], [1, W]])
                    nc.tensor.matmul(
                        ps1[64 * b:64 * b + 64, :],
                        w1s[:, kt, :], rhs,
                        start=(kt == 0), stop=(kt == 1),
                        skip_group_check=(b == 1))
            nc.scalar.activation(
                out=x1q[:].rearrange("p a b -> p (a b)"), in_=ps1[:],
                func=Act.Identity, bias=b1p, scale=1.0)

            # x3q: s-split rep-major channels over the full padded region.
            for ot in range(2):
                ps3 = pshead.tile([128, 1240], f32, tag="psx3", name=f"psx3_{ot}")
                for (o0, n) in ccuts:
                    for kt in range(2):
                        nc.tensor.matmul(
                            ps3[:, o0:o0 + n], w3s[:, kt, ot, :],
                            obn[kt][:, o0:o0 + n],
                            start=(kt == 0), stop=(kt == 1))
                nc.scalar.activation(out=x3q[:, ot, :], in_=ps3[:], func=Act.Copy)

        rsubpp = stack.enter_context(tc.tile_pool(name="rsubpp", bufs=2))
        ysballp = stack.enter_context(tc.tile_pool(name="ysballp", bufs=1))
        ysball = ysballp.tile([128, 2, NP2, 2, W], f32, tag="ysball")
        featp = stack.enter_context(tc.tile_pool(name="featp", bufs=2))
        h2p = stack.enter_context(tc.tile_pool(name="h2p", bufs=1))
        e4p = stack.enter_context(tc.tile_pool(name="e4p", bufs=2))
        prodp = stack.enter_context(tc.tile_pool(name="prodp", bufs=1))
        zinp = stack.enter_context(tc.tile_pool(name="zinp", bufs=2))
        smallp = stack.enter_context(tc.tile_pool(name="smallp", bufs=2))
        ps1p = stack.enter_context(tc.tile_pool(name="ps1p", bufs=2, space="PSUM"))
        ps2p = stack.enter_context(tc.tile_pool(name="ps2p", bufs=2, space="PSUM"))
        paggp = stack.enter_context(tc.tile_pool(name="paggp", bufs=1, space="PSUM"))

        chunk_state = {}

        MB1 = 512     # mm1 psum block cols
        MB2 = 512     # mm2 psum block cols per parity (tile is 2x this)

        def phase1(ci):
            (t0, np_) = CHUNKS[ci]
            nq2 = np_ * W
            vc2 = K2 * nq2

            rsub = rsubpp.tile([4, K2, np_, W], f16, tag="rsub", name=f"rsub{ci}")
            nc.sync.dma_start(out=rsub[:], in_=rsubp_d[:, :, t0:t0 + np_, :])

            feat = featp.tile([128, K2, nq2], f16, tag="feat", name=f"feat{ci}")
            # feat[p, di*7+dj, t, w] = x1q[p, t0+t, w] - x2q[p, 2(t0+t)+di, dj+w]
            x1v = x1q[:]
            x2v = x2q[:]
            fv = feat[:]
            for di in range(K):
                x1w = bass.AP(
                    tensor=x1v.tensor, offset=x1v.offset + t0 * W,
                    ap=[x1v.ap[0], [0, K], [W, np_], [1, W]])
                x2w = bass.AP(
                    tensor=x2v.tensor, offset=x2v.offset + (2 * t0 + di) * WP,
                    ap=[x2v.ap[0], [1, K], [2 * WP, np_], [1, W]])
                outw = bass.AP(
                    tensor=fv.tensor, offset=fv.offset + di * K * nq2,
                    ap=[fv.ap[0], [nq2, K], [W, np_], [1, W]])
                eng = nc.gpsimd if di >= 5 else nc.vector
                eng.tensor_tensor(out=outw, in0=x1w, in1=x2w,
                                  op=Alu.subtract)
            featf = feat[:].rearrange("p a b -> p (a b)")
            nc.vector.tensor_scalar_max(out=featf, in0=featf, scalar1=0.0)

            h2 = h2p.tile([128, vc2], f16, tag="h2")
            e4 = e4p.tile([128, 2, K2, nq2], f16, tag="e4", name=f"e4{ci}")
            rsf = rsub[:].rearrange("p a b c -> p (a b c)")
            del rsub
            for j0 in range(0, vc2, MB1):
                n = min(MB1, vc2 - j0)
                psa = ps1p.tile([128, MB1], f32, tag="ps1")
                for s in range(0, n, 512):
                    sn = min(512, n - s)
                    nc.tensor.matmul(psa[:, s:s + sn], cw1s,
                                     featf[:, j0 + s:j0 + s + sn],
                                     start=True, stop=False)
                    nc.tensor.matmul(psa[:, s:s + sn], posws,
                                     rsf[:, j0 + s:j0 + s + sn],
                                     start=False, stop=True)
                nc.scalar.activation(out=h2[:, j0:j0 + n], in_=psa[:, :n],
                                     func=Act.Relu, bias=b2f, scale=1.0)
            for j0 in range(0, vc2, MB2):
                n = min(MB2, vc2 - j0)
                psb = ps2p.tile([128, 2 * MB2], f32, tag="ps2")
                for par in range(2):
                    nc.tensor.matmul(
                        psb[:, par * MB2:par * MB2 + n],
                        cw2s[64 * par:64 * par + 64, :],
                        h2[64 * par:64 * par + 64, j0:j0 + n],
                        start=True, stop=True)
                e4w = bass.AP(
                    tensor=e4.tensor, offset=e4[:].offset + j0,
                    ap=[e4[:].ap[0], [K2 * nq2, 2], [1, n]])
                psbv = psb[:]
                psin = bass.AP(tensor=psb.tensor, offset=psbv.offset,
                               ap=[psbv.ap[0], [MB2, 2], [1, n]])
                nc.scalar.activation(out=e4w, in_=psin,
                                     func=Act.Exp, bias=cb2r, scale=1.0)
            chunk_state[ci] = (e4,)

        def phase2(ci):
            (t0, np_) = CHUNKS[ci]
            nq2 = np_ * W
            (e4,) = chunk_state[ci]

            # normalizer: tree-sum taps on the (already head-replicated) e4
            zt = zinp.tile([128, 2, 24, nq2], f16, tag="zt", name=f"zt{ci}")
            nc.vector.tensor_tensor(
                out=zt[:], in0=e4[:, :, 0:24, :], in1=e4[:, :, 24:48, :],
                op=Alu.add)
            for (a, b_, n) in [(0, 12, 12), (0, 6, 6), (0, 3, 3)]:
                nc.vector.tensor_tensor(
                    out=zt[:, :, a:a + n, :], in0=zt[:, :, a:a + n, :],
                    in1=zt[:, :, b_:b_ + n, :], op=Alu.add)
            for b_ in (1, 2):
                nc.vector.tensor_tensor(
                    out=zt[:, :, 0, :], in0=zt[:, :, 0, :],
                    in1=zt[:, :, b_, :], op=Alu.add)
            nc.vector.tensor_tensor(
                out=zt[:, :, 0, :], in0=zt[:, :, 0, :],
                in1=e4[:, :, 48, :], op=Alu.add)
            rz = smallp.tile([128, 2, nq2], f16, tag="rz", name=f"rz{ci}")
            with nc.allow_low_precision(reason="fp16 softmax reciprocal"):
                nc.vector.reciprocal(
                    out=rz[:], in_=zt[:, :, 0, :])

            for par in range(2):
                e4v = e4[:, par, :, :]
                prodt = prodp.tile([128, 2, K2, nq2], f16, tag=f"prod{par}",
                                   name=f"prod{par}")
                pv = prodt[:]
                pnum = paggp.tile([128, 2 * nq2], f32, tag="pnum",
                                  name=f"pnum{par}")
                for ot in range(2):
                    for di in range(K):
                        e4w = bass.AP(
                            tensor=e4.tensor,
                            offset=e4v.offset + di * K * nq2,
                            ap=[e4v.ap[0], [nq2, K], [W, np_], [1, W]])
                        x3w = bass.AP(
                            tensor=x3q.tensor,
                            offset=x3q[:].offset + ot * ROWS * WP
                            + (2 * t0 + par + di) * WP,
                            ap=[x3q[:].ap[0], [1, K], [2 * WP, np_], [1, W]])
                        outw = bass.AP(
                            tensor=prodt.tensor,
                            offset=pv.offset + ot * K2 * nq2 + di * K * nq2,
                            ap=[pv.ap[0], [nq2, K], [W, np_], [1, W]])
                        eng = nc.gpsimd if di >= 5 else nc.vector
                        eng.tensor_tensor(out=outw, in0=e4w, in1=x3w,
                                          op=Alu.mult)
                    # numerator: PE identity-accumulation over taps, per ot
                    for k in range(K2):
                        nc.tensor.matmul(
                            pnum[:, ot * nq2:(ot + 1) * nq2], idents,
                            pv[:, ot, k, :],
                            start=(k == 0), stop=(k == K2 - 1),
                            skip_group_check=(ot == 1))

                # divide + bn2/relu + wc + residual
                ob = smallp.tile([128, 2, nq2], f32, tag="ob", name=f"ob{par}")
                rzw = bass.AP(
                    tensor=rz.tensor, offset=rz[:].offset + par * nq2,
                    ap=[rz[:].ap[0], [0, 2], [1, nq2]])
                nc.vector.tensor_tensor(
                    out=ob[:], in0=pnum[:].rearrange("p (a b) -> p a b", a=2),
                    in1=rzw, op=Alu.mult)
                ob2 = smallp.tile([128, 2, nq2], bf16, tag="ob2",
                                  name=f"ob2{par}")
                for ot in range(2):
                    nc.scalar.activation(out=ob2[:, ot, :], in_=ob[:, ot, :],
                                         func=Act.Relu, bias=b3fp[ot],
                                         scale=a3p[ot])
                for oo in range(2):
                    psw = paggp.tile([128, 224], f32, tag="pz",
                                    name=f"pswc{par}{oo}")
                    for kt in range(2):
                        nc.tensor.matmul(psw[:, :nq2], wcs[:, kt, oo, :],
                                         ob2[:, kt, :],
                                         start=(kt == 0), stop=(kt == 1))
                    xi = bass.AP(
                        tensor=xps[oo].tensor,
                        offset=xps[oo][:].offset
                        + (PAD + 2 * t0 + par) * WP + PAD,
                        ap=[xps[oo][:].ap[0], [2 * WP, np_], [1, W]])
                    nc.vector.scalar_tensor_tensor(
                        out=ysball[:, oo, t0:t0 + np_, par, :],
                        in0=psw[:, :nq2].rearrange("p (r w) -> p r w", w=W),
                        scalar=bcb[oo], in1=xi,
                        op0=Alu.add, op1=Alu.add)

        def flush_y(ci):
            (t0, np_) = CHUNKS[ci]
            for oo in range(2):
                nc.sync.dma_start(
                    out=y_d[oo][:, 2 * t0:2 * t0 + 2 * np_, :],
                    in_=ysball[:, oo, t0:t0 + np_, :, :].rearrange(
                        "p t q w -> p (t q) w"))

        phase1(0)
        for ci in range(1, len(CHUNKS)):
            phase1(ci)
            phase2(ci - 1)
            flush_y(ci - 1)
        phase2(len(CHUNKS) - 1)
        flush_y(len(CHUNKS) - 1)

    nc.compile()
    _BUILD_CACHE["nc"] = nc
    return nc


def _host_prep(inputs):
    f = {k: np.asarray(v, np.float32) for k, v in inputs.items()}

    def fold(n):
        a = f[n + "_g"] / np.sqrt(f[n + "_rv"] + EPS)
        return a, f[n + "_b"] - f[n + "_rm"] * a

    a1, b1f = fold("bn1")
    ac, bc1 = fold("cwbn1")
    a2, b2f = fold("cwbn2")
    a3, b3f = fold("bn2")

    W1p = ac[:64, None] * f["w1"]
    b1p = ac[:64] * f["b1"] + bc1[:64]
    W2p = ac[:64, None] * f["w2"]
    b2p = ac[:64] * f["b2"]
    cw1p = a2[:, None] * f["cw1"]          # [64, 66]

    perm = _perm_channels()
    w3p = f["w3"][perm]
    a3p = a3[perm]
    b3fp = b3f[perm]
    rep = np.arange(128) % 32
    cw2r = f["cw2"][rep]
    cb2r = f["cb2"][rep]

    locw = np.tile(np.linspace(-1.0, 1.0, W, dtype=np.float32)[None, :], (H, 1))
    loch = np.tile(np.linspace(-1.0, 1.0, H, dtype=np.float32)[:, None], (1, W))
    loc = np.stack([locw, loch], 0)
    p = np.einsum("chw,oc->ohw", loc, f["pw"]) + f["pb"][:, None, None]
    pp = np.pad(p, ((0, 0), (PAD, PAD), (PAD, PAD)), mode="reflect")
    pu = np.stack([pp[:, i:i + H, j:j + W] for i in range(K) for j in range(K)], 1)
    subp = p[:, None] - pu
    rsubp = np.maximum(ac[64:66, None, None, None] * subp
                       + bc1[64:66, None, None, None], 0).astype(np.float16)

    xpad = np.pad(f["x"], ((0, 0), (0, 0), (PAD, PAD), (PAD, PAD)), mode="reflect")

    w1T = np.ascontiguousarray(W1p.T).reshape(2, 128, 64).copy()
    w2T = np.ascontiguousarray(W2p.T).reshape(2, 128, 64).copy()
    w3T = np.empty((2, 128, 2, 128), np.float32)
    wcT = np.empty((2, 128, 2, 128), np.float32)
    wc_perm = f["wc"][:, perm]
    for kt in range(2):
        for ot in range(2):
            w3T[kt, :, ot, :] = w3p[ot * 128:(ot + 1) * 128,
                                    kt * 128:(kt + 1) * 128].T
            wcT[kt, :, ot, :] = wc_perm[ot * 128:(ot + 1) * 128,
                                        kt * 128:(kt + 1) * 128].T

    cw1T = np.zeros((128, 128), np.float32)
    cw1T[0:64, 0:64] = cw1p[:, 0:64].T
    cw1T[64:128, 64:128] = cw1p[:, 0:64].T
    poswT = np.zeros((4, 128), np.float32)
    poswT[0:2, 0:64] = cw1p[:, 64:66].T
    poswT[2:4, 64:128] = cw1p[:, 64:66].T
    cw2T = np.zeros((128, 128), np.float32)
    cw2T[0:64, :] = cw2r.T
    cw2T[64:128, :] = cw2r.T


    scal = np.zeros((128, 14), np.float32)
    scal[:, 0] = a1[:128]; scal[:, 1] = a1[128:]
    scal[:, 2] = b1f[:128]; scal[:, 3] = b1f[128:]
    scal[:64, 4] = b1p; scal[64:, 4] = b1p
    scal[:64, 5] = b2p; scal[64:, 5] = b2p
    scal[:64, 6] = b2f; scal[64:, 6] = b2f
    scal[:, 7] = cb2r
    scal[:, 8] = a3p[:128]; scal[:, 9] = a3p[128:]
    scal[:, 10] = b3fp[:128]; scal[:, 11] = b3fp[128:]
    scal[:, 12] = f["bc"][:128]; scal[:, 13] = f["bc"][128:]

    wb = np.zeros((128, 1280), np.float32)
    wb[:, 0:64] = w1T[0]; wb[:, 64:128] = w1T[1]
    wb[:, 128:192] = w2T[0]; wb[:, 192:256] = w2T[1]
    wb[:, 256:768] = w3T.transpose(1, 0, 2, 3).reshape(128, 512)
    wb[:, 768:1280] = wcT.transpose(1, 0, 2, 3).reshape(128, 512)
    fbb = np.zeros((128, 640), np.float32)
    fbb[:, 0:128] = cw1T
    fbb[:, 128:256] = cw2T
    fbb[0:4, 256:384] = poswT
    fbb[:, 384:512] = np.eye(128, dtype=np.float32)
    for m in range(4):
        fbb[np.arange(32), 512 + 32 * m + np.arange(32)] = 1.0
    shared = dict(wb=wb.astype(bf16_np), fb=fbb.astype(np.float16),
                  scal=scal)
    in_maps = []
    for core in range(8):
        b, i = divmod(core, 4)
        r0 = RB * i
        m = dict(shared)
        m["xp"] = np.ascontiguousarray(
            xpad[b].reshape(2, 128, H + 2 * PAD, WP)[:, :, r0:r0 + ROWS, :])
        rs = rsubp[:, :, r0:r0 + RB, :]           # [2, 49, 14, 56]
        rs4 = np.empty((4, K2, NP2, W), np.float16)
        rs4[0:2] = rs[:, :, 0::2, :]
        rs4[2:4] = rs[:, :, 1::2, :]
        m["rsubp"] = np.ascontiguousarray(rs4)
        in_maps.append(m)
    return in_maps


def kernel(**inputs):
    from concourse.bass_utils import run_bass_kernel_spmd
    nc = _build_program()
    in_maps = _host_prep(inputs)
    res = run_bass_kernel_spmd(nc, in_maps, core_ids=list(range(8)))
    global LAST_RESULTS
    LAST_RESULTS = res
    y = np.zeros((B, C, H, W), np.float32)
    for core in range(8):
        b, i = divmod(core, 4)
        yc = res.results[core]["y"]
        y[b, :, RB * i:RB * (i + 1), :] = yc.reshape(C, RB, W)
    return y


# revision 58
# speedup vs baseline: 1.4828x; 1.0189x over previous
"""SAN Bottleneck (pairwise self-attention) Trainium2 kernel.

Sharding: 8 cores = 2 batches x 4 row-blocks of 14 rows (H=56). Each core
receives a reflect-padded input slice (20 rows x 62 cols), so the 7x7
unfold needs no runtime halo exchange and no edge special-casing.

Per-core pipeline (batchnorms folded into per-channel scale/bias on host):
  bn1+relu -> x1/x2/x3 1x1 convs (bf16 matmuls).
  Query rows are PARITY-PACKED: partitions 0:64 = feature channels for
  even query rows, 64:128 = odd rows. x1q/x2q are produced directly in
  that layout (x2q's odd block is the same conv shifted one padded row),
  so feat = relu(x1 - win(x2)) runs at full 128-partition width and the
  conv_w matmul chain does half the per-partition work.
  mm1 is one 128-contraction block-diagonal matmul; the position branch
  accumulates into the same PSUM via a 4-partition matmul.
  mm2 (64->128, head weights replicated rep-major: partition p = head
  p%32) produces e4 = exp(logits) for both parities in one PSUM evac.
  Aggregation: per-tap window products vs x3 (s-split rep-major channels)
  + in-place pairwise tree sums, split across DVE and GPSIMD (GPSIMD ops
  issued as scalar_tensor_tensor).
  Softmax normalizer: e4's head rows are DMA-packed 4 query-quarters wide
  [128 = 4q x 32h], tree-summed at 1/4 cost, DMA-unpacked to [32, nq],
  replicated across partition blocks by a 0/1 matmul, reciprocal, divide.
  bn2+relu -> wc conv + bias + identity residual.
"""

import numpy as np
import ml_dtypes

bf16_np = ml_dtypes.bfloat16

K = 7
PAD = 3
EPS = 1e-5
B, C, H, W = 2, 256, 56, 56
RB = 14              # rows per core
ROWS = RB + 2 * PAD  # 20
WP = W + 2 * PAD     # 62
K2 = K * K
NP2 = RB // 2        # 7 row-pairs
CHUNKS = [(0, 2), (2, 2), (4, 2), (6, 1)]   # (row-pair offset, row-pairs)

_BUILD_CACHE = {}


def _perm_channels():
    # rep-major s-split: partition p of ot tile t <-> channel 8*(p%32)+4t+(p//32)
    perm = np.zeros(256, np.int64)
    for t in range(2):
        for p in range(128):
            perm[t * 128 + p] = 8 * (p % 32) + 4 * t + (p // 32)
    return perm


def _build_program():
    if "nc" in _BUILD_CACHE:
        return _BUILD_CACHE["nc"]
    import concourse.bass as bass
    import concourse.bacc as bacc
    import concourse.tile as tile
    import concourse.mybir as mybir
    from contextlib import ExitStack

    f32 = mybir.dt.float32
    f16 = mybir.dt.float16
    bf16 = mybir.dt.bfloat16
    Alu = mybir.AluOpType
    Act = mybir.ActivationFunctionType

    nc = bacc.Bacc("TRN2", target_bir_lowering=False, num_devices=8)

    xp_d = nc.dram_tensor("xp", [2, 128, ROWS, WP], f32, kind="ExternalInput")
    rsubp_d = nc.dram_tensor("rsubp", [4, K2, NP2, W], f16, kind="ExternalInput")
    wb_d = nc.dram_tensor("wb", [128, 1280], bf16, kind="ExternalInput")
    fb_d = nc.dram_tensor("fb", [128, 640], f16, kind="ExternalInput")
    scal_d = nc.dram_tensor("scal", [128, 14], f32, kind="ExternalInput")
    y_d = nc.dram_tensor("y", [2, 128, RB, W], f32, kind="ExternalOutput")

    with tile.TileContext(nc) as tc, ExitStack() as stack:
        consts = stack.enter_context(tc.tile_pool(name="consts", bufs=1))
        xpp = stack.enter_context(tc.tile_pool(name="xpp", bufs=1))
        headsb = stack.enter_context(tc.tile_pool(name="headsb", bufs=1))

        wb = consts.tile([128, 1280], bf16, tag="wb")
        fb = consts.tile([128, 640], f16, tag="fb")
        scals = consts.tile([128, 14], f32, tag="scals")
        nc.sync.dma_start(out=scals[:], in_=scal_d[:])
        w1s = wb[:, 0:128].rearrange("p (k a) -> p k a", k=2)
        w2s = wb[:, 128:256].rearrange("p (k a) -> p k a", k=2)
        w3s = wb[:, 256:768].rearrange("p (k o a) -> p k o a", k=2, o=2)
        wcs = wb[:, 768:1280].rearrange("p (k o a) -> p k o a", k=2, o=2)
        cw1s = fb[:, 0:128]
        cw2s = fb[:, 128:256]
        posws = fb[0:4, 256:384]
        idents = fb[:, 384:512]
        repms = fb[0:32, 512:640]

        a1 = [scals[:, 0:1], scals[:, 1:2]]
        b1f = [scals[:, 2:3], scals[:, 3:4]]
        b1p = scals[:, 4:5]
        b2p = scals[:, 5:6]
        b2f = scals[:, 6:7]
        cb2r = scals[:, 7:8]
        a3p = [scals[:, 8:9], scals[:, 9:10]]
        b3fp = [scals[:, 10:11], scals[:, 11:12]]
        bcb = [scals[:, 12:13], scals[:, 13:14]]

        xps = [xpp.tile([128, ROWS, WP], f32, tag=f"xp{t}", name=f"xp{t}")
               for t in range(2)]
        nc.sync.dma_start(out=xps[0][:], in_=xp_d[0])
        nc.sync.dma_start(out=wb[:], in_=wb_d[:])
        nc.sync.dma_start(out=xps[1][:, 0:10, :], in_=xp_d[1][:, 0:10, :])
        nc.sync.dma_start(out=xps[1][:, 10:20, :], in_=xp_d[1][:, 10:20, :])
        nc.sync.dma_start(out=fb[:], in_=fb_d[:])
        # PE warmup: keep the PE busy during input load so the clock gate
        # opens before the head convolutions.
        warm = consts.tile([128, 16], f16, tag="warm")
        nc.vector.memset(warm[:], 0.0)
        warma = consts.tile([128, 16], f16, tag="warma")
        nc.scalar.activation(out=warma[:], in_=warm[:], func=Act.Exp,
                             bias=0.0, scale=1.0)
        obn = [headsb.tile([128, ROWS * WP], bf16, tag=f"obn{t}", name=f"obn{t}")
               for t in range(2)]
        for (h0, h1) in ((0, 620), (620, 1240)):
            nc.scalar.activation(
                out=obn[0][:, h0:h1],
                in_=xps[0][:].rearrange("p r w -> p (r w)")[:, h0:h1],
                func=Act.Relu, bias=b1f[0], scale=a1[0])
            nc.vector.tensor_scalar(
                out=obn[1][:, h0:h1],
                in0=xps[1][:].rearrange("p r w -> p (r w)")[:, h0:h1],
                scalar1=a1[1], scalar2=b1f[1], op0=Alu.mult, op1=Alu.add)
            nc.vector.tensor_scalar_max(out=obn[1][:, h0:h1],
                                        in0=obn[1][:, h0:h1], scalar1=0.0)

        x1q = headsb.tile([128, NP2, W], f16, tag="x1q")
        x2q = headsb.tile([128, ROWS * WP], f16, tag="x2q")
        x3q = headsb.tile([128, 2, ROWS * WP], f16, tag="x3q")

        ccuts = [(0, 512), (512, 512), (1024, 216)]
        with tc.tile_pool(name="pshead", bufs=1, space="PSUM") as pshead:
            pwarm = pshead.tile([128, 512], f32, tag="pwarm")
            wrhs = bass.AP(tensor=warm.tensor, offset=warm[:].offset,
                           ap=[warm[:].ap[0], [0, 32], [1, 16]])
            for wi in range(8):
                nc.tensor.matmul(pwarm[0:16, :], warm[:], wrhs,
                                 start=True, stop=True)
            # x2q: partition block b = x2 over padded rows shifted by b.
            ps = pshead.tile([128, 1240], f32, tag="psx2")
            for (o0, n) in ccuts:
                mms = []
                for b in range(2):
                    lo = b * WP
                    nn_ = max(0, min(n, ROWS * WP - lo - o0))
                    if nn_ == 0:
                        continue
                    for kt in range(2):
                        mms.append((b, kt, nn_, lo))
                for (b, kt, nn_, lo) in mms:
                    nc.tensor.matmul(
                        ps[64 * b:64 * b + 64, o0:o0 + nn_],
                        w2s[:, kt, :],
                        obn[kt][:, lo + o0:lo + o0 + nn_],
                        start=(kt == 0), stop=(kt == 1),
                        skip_group_check=(b == 1))
            nc.scalar.activation(out=x2q[:, 0:682], in_=ps[:, 0:682],
                                 func=Act.Identity, bias=b2p, scale=1.0)
            nc.scalar.activation(out=x2q[:, 682:1178], in_=ps[:, 682:1178],
                                 func=Act.Identity, bias=b2p, scale=1.0)
            nc.scalar.activation(out=x2q[0:64, 1178:1240],
                                 in_=ps[0:64, 1178:1240],
                                 func=Act.Identity, bias=scals[0:64, 5:6],
                                 scale=1.0)

            # x1q: partition block b = x1 at query rows 2t+b.
            ps1 = pshead.tile([128, NP2 * W], f32, tag="psx1")
            for b in range(2):
                for kt in range(2):
                    ob = obn[kt][:].rearrange("p (r w) -> p r w", w=WP)
                    rhs = bass.AP(
                        tensor=ob.tensor,
                        offset=ob.offset + (PAD + b) * WP + PAD,
                        ap=[ob.ap[0], [2 * WP, NP2], [1, W]])
                    nc.tensor.matmul(
                        ps1[64 * b:64 * b + 64, :],
                        w1s[:, kt, :], rhs,
                        start=(kt == 0), stop=(kt == 1),
                        skip_group_check=(b == 1))
            nc.scalar.activation(
                out=x1q[:].rearrange("p a b -> p (a b)"), in_=ps1[:],
                func=Act.Identity, bias=b1p, scale=1.0)

            # x3q: s-split rep-major channels over the full padded region.
            for ot in range(2):
                ps3 = pshead.tile([128, 1240], f32, tag="psx3", name=f"psx3_{ot}")
                for (o0, n) in ccuts:
                    for kt in range(2):
                        nc.tensor.matmul(
                            ps3[:, o0:o0 + n], w3s[:, kt, ot, :],
                            obn[kt][:, o0:o0 + n],
                            start=(kt == 0), stop=(kt == 1))
                nc.scalar.activation(out=x3q[:, ot, :], in_=ps3[:], func=Act.Copy)

        rsubpp = stack.enter_context(tc.tile_pool(name="rsubpp", bufs=2))
        ysballp = stack.enter_context(tc.tile_pool(name="ysballp", bufs=1))
        ysball = ysballp.tile([128, 2, NP2, 2, W], f32, tag="ysball")
        featp = stack.enter_context(tc.tile_pool(name="featp", bufs=2))
        h2p = stack.enter_context(tc.tile_pool(name="h2p", bufs=1))
        e4p = stack.enter_context(tc.tile_pool(name="e4p", bufs=2))
        prodp = stack.enter_context(tc.tile_pool(name="prodp", bufs=1))
        zinp = stack.enter_context(tc.tile_pool(name="zinp", bufs=2))
        smallp = stack.enter_context(tc.tile_pool(name="smallp", bufs=2))
        ps1p = stack.enter_context(tc.tile_pool(name="ps1p", bufs=2, space="PSUM"))
        ps2p = stack.enter_context(tc.tile_pool(name="ps2p", bufs=2, space="PSUM"))
        paggp = stack.enter_context(tc.tile_pool(name="paggp", bufs=1, space="PSUM"))

        chunk_state = {}

        MB1 = 512     # mm1 psum block cols
        MB2 = 512     # mm2 psum block cols per parity (tile is 2x this)

        def phase1(ci):
            (t0, np_) = CHUNKS[ci]
            nq2 = np_ * W
            vc2 = K2 * nq2

            rsub = rsubpp.tile([4, K2, np_, W], f16, tag="rsub", name=f"rsub{ci}")
            nc.sync.dma_start(out=rsub[:], in_=rsubp_d[:, :, t0:t0 + np_, :])

            feat = featp.tile([128, K2, nq2], f16, tag="feat", name=f"feat{ci}")
            # feat[p, di*7+dj, t, w] = x1q[p, t0+t, w] - x2q[p, 2(t0+t)+di, dj+w]
            x1v = x1q[:]
            x2v = x2q[:]
            fv = feat[:]
            for di in range(K):
                x1w = bass.AP(
                    tensor=x1v.tensor, offset=x1v.offset + t0 * W,
                    ap=[x1v.ap[0], [0, K], [W, np_], [1, W]])
                x2w = bass.AP(
                    tensor=x2v.tensor, offset=x2v.offset + (2 * t0 + di) * WP,
                    ap=[x2v.ap[0], [1, K], [2 * WP, np_], [1, W]])
                outw = bass.AP(
                    tensor=fv.tensor, offset=fv.offset + di * K * nq2,
                    ap=[fv.ap[0], [nq2, K], [W, np_], [1, W]])
                eng = nc.gpsimd if di >= 5 else nc.vector
                eng.tensor_tensor(out=outw, in0=x1w, in1=x2w,
                                  op=Alu.subtract)
            featf = feat[:].rearrange("p a b -> p (a b)")
            nc.vector.tensor_scalar_max(out=featf, in0=featf, scalar1=0.0)

            h2 = h2p.tile([128, vc2], f16, tag="h2")
            e4 = e4p.tile([128, 2, K2, nq2], f16, tag="e4", name=f"e4{ci}")
            rsf = rsub[:].rearrange("p a b c -> p (a b c)")
            del rsub
            for j0 in range(0, vc2, MB1):
                n = min(MB1, vc2 - j0)
                psa = ps1p.tile([128, MB1], f32, tag="ps1")
                for s in range(0, n, 512):
                    sn = min(512, n - s)
                    nc.tensor.matmul(psa[:, s:s + sn], cw1s,
                                     featf[:, j0 + s:j0 + s + sn],
                                     start=True, stop=False)
                    nc.tensor.matmul(psa[:, s:s + sn], posws,
                                     rsf[:, j0 + s:j0 + s + sn],
                                     start=False, stop=True)
                nc.scalar.activation(out=h2[:, j0:j0 + n], in_=psa[:, :n],
                                     func=Act.Relu, bias=b2f, scale=1.0)
            for j0 in range(0, vc2, MB2):
                n = min(MB2, vc2 - j0)
                psb = ps2p.tile([128, 2 * MB2], f32, tag="ps2")
                for par in range(2):
                    nc.tensor.matmul(
                        psb[:, par * MB2:par * MB2 + n],
                        cw2s[64 * par:64 * par + 64, :],
                        h2[64 * par:64 * par + 64, j0:j0 + n],
                        start=True, stop=True)
                e4w = bass.AP(
                    tensor=e4.tensor, offset=e4[:].offset + j0,
                    ap=[e4[:].ap[0], [K2 * nq2, 2], [1, n]])
                psbv = psb[:]
                psin = bass.AP(tensor=psb.tensor, offset=psbv.offset,
                               ap=[psbv.ap[0], [MB2, 2], [1, n]])
                nc.scalar.activation(out=e4w, in_=psin,
                                     func=Act.Exp, bias=cb2r, scale=1.0)
            chunk_state[ci] = (e4,)

        def phase2(ci):
            (t0, np_) = CHUNKS[ci]
            nq2 = np_ * W
            (e4,) = chunk_state[ci]

            # normalizer: tree-sum taps on the (already head-replicated) e4
            zt = zinp.tile([128, 2, 24, nq2], f16, tag="zt", name=f"zt{ci}")
            nc.vector.tensor_tensor(
                out=zt[:], in0=e4[:, :, 0:24, :], in1=e4[:, :, 24:48, :],
                op=Alu.add)
            for (a, b_, n) in [(0, 12, 12), (0, 6, 6), (0, 3, 3)]:
                nc.vector.tensor_tensor(
                    out=zt[:, :, a:a + n, :], in0=zt[:, :, a:a + n, :],
                    in1=zt[:, :, b_:b_ + n, :], op=Alu.add)
            for b_ in (1, 2):
                nc.vector.tensor_tensor(
                    out=zt[:, :, 0, :], in0=zt[:, :, 0, :],
                    in1=zt[:, :, b_, :], op=Alu.add)
            nc.vector.tensor_tensor(
                out=zt[:, :, 0, :], in0=zt[:, :, 0, :],
                in1=e4[:, :, 48, :], op=Alu.add)
            rz = smallp.tile([128, 2, nq2], f16, tag="rz", name=f"rz{ci}")
            with nc.allow_low_precision(reason="fp16 softmax reciprocal"):
                nc.vector.reciprocal(
                    out=rz[:], in_=zt[:, :, 0, :])

            for par in range(2):
                e4v = e4[:, par, :, :]
                prodt = prodp.tile([128, 2, K2, nq2], f16, tag=f"prod{par}",
                                   name=f"prod{par}")
                pv = prodt[:]
                pnum = paggp.tile([128, 2 * nq2], f32, tag="pnum",
                                  name=f"pnum{par}")
                POOL_DI = {0: (5, 6), 1: (5, 6)}   # ot -> pool di set
                for ot in range(2):
                    pool_di = POOL_DI[ot]
                    for di in list(pool_di) + [d for d in range(K)
                                               if d not in pool_di]:
                        e4w = bass.AP(
                            tensor=e4.tensor,
                            offset=e4v.offset + di * K * nq2,
                            ap=[e4v.ap[0], [nq2, K], [W, np_], [1, W]])
                        x3w = bass.AP(
                            tensor=x3q.tensor,
                            offset=x3q[:].offset + ot * ROWS * WP
                            + (2 * t0 + par + di) * WP,
                            ap=[x3q[:].ap[0], [1, K], [2 * WP, np_], [1, W]])
                        outw = bass.AP(
                            tensor=prodt.tensor,
                            offset=pv.offset + ot * K2 * nq2 + di * K * nq2,
                            ap=[pv.ap[0], [nq2, K], [W, np_], [1, W]])
                        eng = nc.gpsimd if di in pool_di else nc.vector
                        eng.tensor_tensor(out=outw, in0=e4w, in1=x3w,
                                          op=Alu.mult)
                    # numerator: PE identity-accumulation over taps, per ot.
                    # DVE-produced taps first; slower GPSIMD taps last.
                    korder = ([k for k in range(K2) if k // K not in pool_di]
                              + [k for k in range(K2) if k // K in pool_di])
                    for i, k in enumerate(korder):
                        nc.tensor.matmul(
                            pnum[:, ot * nq2:(ot + 1) * nq2], idents,
                            pv[:, ot, k, :],
                            start=(i == 0), stop=(i == K2 - 1),
                            skip_group_check=(ot == 1))

                # divide + bn2/relu + wc + residual
                ob = smallp.tile([128, 2, nq2], f32, tag="ob", name=f"ob{par}")
                rzw = bass.AP(
                    tensor=rz.tensor, offset=rz[:].offset + par * nq2,
                    ap=[rz[:].ap[0], [0, 2], [1, nq2]])
                nc.vector.tensor_tensor(
                    out=ob[:], in0=pnum[:].rearrange("p (a b) -> p a b", a=2),
                    in1=rzw, op=Alu.mult)
                ob2 = smallp.tile([128, 2, nq2], bf16, tag="ob2",
                                  name=f"ob2{par}")
                for ot in range(2):
                    nc.vector.tensor_scalar(
                        out=ob2[:, ot, :], in0=ob[:, ot, :],
                        scalar1=a3p[ot], scalar2=b3fp[ot],
                        op0=Alu.mult, op1=Alu.add)
                    nc.vector.tensor_scalar_max(
                        out=ob2[:, ot, :], in0=ob2[:, ot, :], scalar1=0.0)
                for oo in range(2):
                    psw = paggp.tile([128, 224], f32, tag="pz",
                                    name=f"pswc{par}{oo}")
                    for kt in range(2):
                        nc.tensor.matmul(psw[:, :nq2], wcs[:, kt, oo, :],
                                         ob2[:, kt, :],
                                         start=(kt == 0), stop=(kt == 1))
                    xi = bass.AP(
                        tensor=xps[oo].tensor,
                        offset=xps[oo][:].offset
                        + (PAD + 2 * t0 + par) * WP + PAD,
                        ap=[xps[oo][:].ap[0], [2 * WP, np_], [1, W]])
                    nc.vector.scalar_tensor_tensor(
                        out=ysball[:, oo, t0:t0 + np_, par, :],
                        in0=psw[:, :nq2].rearrange("p (r w) -> p r w", w=W),
                        scalar=bcb[oo], in1=xi,
                        op0=Alu.add, op1=Alu.add)

        def flush_y(ci):
            (t0, np_) = CHUNKS[ci]
            for oo in range(2):
                nc.sync.dma_start(
                    out=y_d[oo][:, 2 * t0:2 * t0 + 2 * np_, :],
                    in_=ysball[:, oo, t0:t0 + np_, :, :].rearrange(
                        "p t q w -> p (t q) w"))

        phase1(0)
        for ci in range(1, len(CHUNKS)):
            phase1(ci)
            phase2(ci - 1)
            flush_y(ci - 1)
        phase2(len(CHUNKS) - 1)
        flush_y(len(CHUNKS) - 1)

    nc.compile()
    _BUILD_CACHE["nc"] = nc
    return nc


def _host_prep(inputs):
    f = {k: np.asarray(v, np.float32) for k, v in inputs.items()}

    def fold(n):
        a = f[n + "_g"] / np.sqrt(f[n + "_rv"] + EPS)
        return a, f[n + "_b"] - f[n + "_rm"] * a

    a1, b1f = fold("bn1")
    ac, bc1 = fold("cwbn1")
    a2, b2f = fold("cwbn2")
    a3, b3f = fold("bn2")

    W1p = ac[:64, None] * f["w1"]
    b1p = ac[:64] * f["b1"] + bc1[:64]
    W2p = ac[:64, None] * f["w2"]
    b2p = ac[:64] * f["b2"]
    cw1p = a2[:, None] * f["cw1"]          # [64, 66]

    perm = _perm_channels()
    w3p = f["w3"][perm]
    a3p = a3[perm]
    b3fp = b3f[perm]
    rep = np.arange(128) % 32
    cw2r = f["cw2"][rep]
    cb2r = f["cb2"][rep]

    locw = np.tile(np.linspace(-1.0, 1.0, W, dtype=np.float32)[None, :], (H, 1))
    loch = np.tile(np.linspace(-1.0, 1.0, H, dtype=np.float32)[:, None], (1, W))
    loc = np.stack([locw, loch], 0)
    p = np.einsum("chw,oc->ohw", loc, f["pw"]) + f["pb"][:, None, None]
    pp = np.pad(p, ((0, 0), (PAD, PAD), (PAD, PAD)), mode="reflect")
    pu = np.stack([pp[:, i:i + H, j:j + W] for i in range(K) for j in range(K)], 1)
    subp = p[:, None] - pu
    rsubp = np.maximum(ac[64:66, None, None, None] * subp
                       + bc1[64:66, None, None, None], 0).astype(np.float16)

    xpad = np.pad(f["x"], ((0, 0), (0, 0), (PAD, PAD), (PAD, PAD)), mode="reflect")

    w1T = np.ascontiguousarray(W1p.T).reshape(2, 128, 64).copy()
    w2T = np.ascontiguousarray(W2p.T).reshape(2, 128, 64).copy()
    w3T = np.empty((2, 128, 2, 128), np.float32)
    wcT = np.empty((2, 128, 2, 128), np.float32)
    wc_perm = f["wc"][:, perm]
    for kt in range(2):
        for ot in range(2):
            w3T[kt, :, ot, :] = w3p[ot * 128:(ot + 1) * 128,
                                    kt * 128:(kt + 1) * 128].T
            wcT[kt, :, ot, :] = wc_perm[ot * 128:(ot + 1) * 128,
                                        kt * 128:(kt + 1) * 128].T

    cw1T = np.zeros((128, 128), np.float32)
    cw1T[0:64, 0:64] = cw1p[:, 0:64].T
    cw1T[64:128, 64:128] = cw1p[:, 0:64].T
    poswT = np.zeros((4, 128), np.float32)
    poswT[0:2, 0:64] = cw1p[:, 64:66].T
    poswT[2:4, 64:128] = cw1p[:, 64:66].T
    cw2T = np.zeros((128, 128), np.float32)
    cw2T[0:64, :] = cw2r.T
    cw2T[64:128, :] = cw2r.T


    scal = np.zeros((128, 14), np.float32)
    scal[:, 0] = a1[:128]; scal[:, 1] = a1[128:]
    scal[:, 2] = b1f[:128]; scal[:, 3] = b1f[128:]
    scal[:64, 4] = b1p; scal[64:, 4] = b1p
    scal[:64, 5] = b2p; scal[64:, 5] = b2p
    scal[:64, 6] = b2f; scal[64:, 6] = b2f
    scal[:, 7] = cb2r
    scal[:, 8] = a3p[:128]; scal[:, 9] = a3p[128:]
    scal[:, 10] = b3fp[:128]; scal[:, 11] = b3fp[128:]
    scal[:, 12] = f["bc"][:128]; scal[:, 13] = f["bc"][128:]

    wb = np.zeros((128, 1280), np.float32)
    wb[:, 0:64] = w1T[0]; wb[:, 64:128] = w1T[1]
    wb[:, 128:192] = w2T[0]; wb[:, 192:256] = w2T[1]
    wb[:, 256:768] = w3T.transpose(1, 0, 2, 3).reshape(128, 512)
    wb[:, 768:1280] = wcT.transpose(1, 0, 2, 3).reshape(128, 512)
    fbb = np.zeros((128, 640), np.float32)
    fbb[:, 0:128] = cw1T
    fbb[:, 128:256] = cw2T
    fbb[0:4, 256:384] = poswT
    fbb[:, 384:512] = np.eye(128, dtype=np.float32)
    for m in range(4):
        fbb[np.arange(32), 512 + 32 * m + np.arange(32)] = 1.0
    shared = dict(wb=wb.astype(bf16_np), fb=fbb.astype(np.float16),
                  scal=scal)
    in_maps = []
    for core in range(8):
        b, i = divmod(core, 4)
        r0 = RB * i
        m = dict(shared)
        m["xp"] = np.ascontiguousarray(
            xpad[b].reshape(2, 128, H + 2 * PAD, WP)[:, :, r0:r0 + ROWS, :])
        rs = rsubp[:, :, r0:r0 + RB, :]           # [2, 49, 14, 56]
        rs4 = np.empty((4, K2, NP2, W), np.float16)
        rs4[0:2] = rs[:, :, 0::2, :]
        rs4[2:4] = rs[:, :, 1::2, :]
        m["rsubp"] = np.ascontiguousarray(rs4)
        in_maps.append(m)
    return in_maps


def kernel(**inputs):
    from concourse.bass_utils import run_bass_kernel_spmd
    nc = _build_program()
    in_maps = _host_prep(inputs)
    res = run_bass_kernel_spmd(nc, in_maps, core_ids=list(range(8)))
    global LAST_RESULTS
    LAST_RESULTS = res
    y = np.zeros((B, C, H, W), np.float32)
    for core in range(8):
        b, i = divmod(core, 4)
        yc = res.results[core]["y"]
        y[b, :, RB * i:RB * (i + 1), :] = yc.reshape(C, RB, W)
    return y


# revision 67
# speedup vs baseline: 1.4853x; 1.0016x over previous
"""SAN Bottleneck (pairwise self-attention) Trainium2 kernel.

Sharding: 8 cores = 2 batches x 4 row-blocks of 14 rows (H=56). Each core
receives a reflect-padded input slice (20 rows x 62 cols), so the 7x7
unfold needs no runtime halo exchange and no edge special-casing.

Per-core pipeline (batchnorms folded into per-channel scale/bias on host):
  bn1+relu -> x1/x2/x3 1x1 convs (bf16 matmuls).
  Query rows are PARITY-PACKED: partitions 0:64 = feature channels for
  even query rows, 64:128 = odd rows. x1q/x2q are produced directly in
  that layout (x2q's odd block is the same conv shifted one padded row),
  so feat = relu(x1 - win(x2)) runs at full 128-partition width and the
  conv_w matmul chain does half the per-partition work.
  mm1 is one 128-contraction block-diagonal matmul; the position branch
  accumulates into the same PSUM via a 4-partition matmul.
  mm2 (64->128, head weights replicated rep-major: partition p = head
  p%32) produces e4 = exp(logits) for both parities in one PSUM evac.
  Aggregation: per-tap window products vs x3 (s-split rep-major channels)
  + in-place pairwise tree sums, split across DVE and GPSIMD (GPSIMD ops
  issued as scalar_tensor_tensor).
  Softmax normalizer: e4's head rows are DMA-packed 4 query-quarters wide
  [128 = 4q x 32h], tree-summed at 1/4 cost, DMA-unpacked to [32, nq],
  replicated across partition blocks by a 0/1 matmul, reciprocal, divide.
  bn2+relu -> wc conv + bias + identity residual.
"""

import numpy as np
import ml_dtypes

bf16_np = ml_dtypes.bfloat16

K = 7
PAD = 3
EPS = 1e-5
B, C, H, W = 2, 256, 56, 56
RB = 14              # rows per core
ROWS = RB + 2 * PAD  # 20
WP = W + 2 * PAD     # 62
K2 = K * K
NP2 = RB // 2        # 7 row-pairs
CHUNKS = [(0, 2), (2, 2), (4, 2), (6, 1)]   # (row-pair offset, row-pairs)

_BUILD_CACHE = {}


def _perm_channels():
    # rep-major s-split: partition p of ot tile t <-> channel 8*(p%32)+4t+(p//32)
    perm = np.zeros(256, np.int64)
    for t in range(2):
        for p in range(128):
            perm[t * 128 + p] = 8 * (p % 32) + 4 * t + (p // 32)
    return perm


def _build_program():
    if "nc" in _BUILD_CACHE:
        return _BUILD_CACHE["nc"]
    import concourse.bass as bass
    import concourse.bacc as bacc
    import concourse.tile as tile
    import concourse.mybir as mybir
    from contextlib import ExitStack

    f32 = mybir.dt.float32
    f16 = mybir.dt.float16
    bf16 = mybir.dt.bfloat16
    Alu = mybir.AluOpType
    Act = mybir.ActivationFunctionType

    nc = bacc.Bacc("TRN2", target_bir_lowering=False, num_devices=8)

    xp_d = nc.dram_tensor("xp", [2, 128, ROWS, WP], f32, kind="ExternalInput")
    rsubp_d = nc.dram_tensor("rsubp", [4, K2, NP2, W], f16, kind="ExternalInput")
    wb_d = nc.dram_tensor("wb", [128, 1280], bf16, kind="ExternalInput")
    fb_d = nc.dram_tensor("fb", [128, 640], f16, kind="ExternalInput")
    scal_d = nc.dram_tensor("scal", [128, 14], f32, kind="ExternalInput")
    y_d = nc.dram_tensor("y", [2, 128, RB, W], f32, kind="ExternalOutput")

    with tile.TileContext(nc) as tc, ExitStack() as stack:
        consts = stack.enter_context(tc.tile_pool(name="consts", bufs=1))
        xpp = stack.enter_context(tc.tile_pool(name="xpp", bufs=1))
        headsb = stack.enter_context(tc.tile_pool(name="headsb", bufs=1))

        wb = consts.tile([128, 1280], bf16, tag="wb")
        fb = consts.tile([128, 640], f16, tag="fb")
        scals = consts.tile([128, 14], f32, tag="scals")
        nc.sync.dma_start(out=scals[:], in_=scal_d[:])
        w1s = wb[:, 0:128].rearrange("p (k a) -> p k a", k=2)
        w2s = wb[:, 128:256].rearrange("p (k a) -> p k a", k=2)
        w3s = wb[:, 256:768].rearrange("p (k o a) -> p k o a", k=2, o=2)
        wcs = wb[:, 768:1280].rearrange("p (k o a) -> p k o a", k=2, o=2)
        cw1s = fb[:, 0:128]
        cw2s = fb[:, 128:256]
        posws = fb[0:4, 256:384]
        idents = fb[:, 384:512]
        repms = fb[0:32, 512:640]

        a1 = [scals[:, 0:1], scals[:, 1:2]]
        b1f = [scals[:, 2:3], scals[:, 3:4]]
        b1p = scals[:, 4:5]
        b2p = scals[:, 5:6]
        b2f = scals[:, 6:7]
        cb2r = scals[:, 7:8]
        a3p = [scals[:, 8:9], scals[:, 9:10]]
        b3fp = [scals[:, 10:11], scals[:, 11:12]]
        bcb = [scals[:, 12:13], scals[:, 13:14]]

        xps = [xpp.tile([128, ROWS, WP], f32, tag=f"xp{t}", name=f"xp{t}")
               for t in range(2)]
        nc.sync.dma_start(out=xps[0][:], in_=xp_d[0])
        nc.sync.dma_start(out=wb[:], in_=wb_d[:])
        nc.sync.dma_start(out=xps[1][:, 0:10, :], in_=xp_d[1][:, 0:10, :])
        nc.sync.dma_start(out=xps[1][:, 10:20, :], in_=xp_d[1][:, 10:20, :])
        nc.sync.dma_start(out=fb[:], in_=fb_d[:])
        # PE warmup: keep the PE busy during input load so the clock gate
        # opens before the head convolutions.
        warm = consts.tile([128, 16], f16, tag="warm")
        nc.vector.memset(warm[:], 0.0)
        warma = consts.tile([128, 16], f16, tag="warma")
        nc.scalar.activation(out=warma[:], in_=warm[:], func=Act.Exp,
                             bias=0.0, scale=1.0)
        obn = [headsb.tile([128, ROWS * WP], bf16, tag=f"obn{t}", name=f"obn{t}")
               for t in range(2)]
        for (h0, h1) in ((0, 620), (620, 1240)):
            nc.scalar.activation(
                out=obn[0][:, h0:h1],
                in_=xps[0][:].rearrange("p r w -> p (r w)")[:, h0:h1],
                func=Act.Relu, bias=b1f[0], scale=a1[0])
            nc.vector.tensor_scalar(
                out=obn[1][:, h0:h1],
                in0=xps[1][:].rearrange("p r w -> p (r w)")[:, h0:h1],
                scalar1=a1[1], scalar2=b1f[1], op0=Alu.mult, op1=Alu.add)
            nc.vector.tensor_scalar_max(out=obn[1][:, h0:h1],
                                        in0=obn[1][:, h0:h1], scalar1=0.0)

        x1q = headsb.tile([128, NP2, W], f16, tag="x1q")
        x2q = headsb.tile([128, ROWS * WP], f16, tag="x2q")
        x3q = headsb.tile([128, 2, ROWS * WP], f16, tag="x3q")

        ccuts = [(0, 512), (512, 512), (1024, 216)]
        with tc.tile_pool(name="pshead", bufs=1, space="PSUM") as pshead:
            pwarm = pshead.tile([128, 512], f32, tag="pwarm")
            wrhs = bass.AP(tensor=warm.tensor, offset=warm[:].offset,
                           ap=[warm[:].ap[0], [0, 32], [1, 16]])
            for wi in range(8):
                nc.tensor.matmul(pwarm[0:16, :], warm[:], wrhs,
                                 start=True, stop=True)
            # x2q: partition block b = x2 over padded rows shifted by b.
            ps = pshead.tile([128, 1240], f32, tag="psx2")
            for (o0, n) in ccuts:
                mms = []
                for b in range(2):
                    lo = b * WP
                    nn_ = max(0, min(n, ROWS * WP - lo - o0))
                    if nn_ == 0:
                        continue
                    for kt in range(2):
                        mms.append((b, kt, nn_, lo))
                for (b, kt, nn_, lo) in mms:
                    nc.tensor.matmul(
                        ps[64 * b:64 * b + 64, o0:o0 + nn_],
                        w2s[:, kt, :],
                        obn[kt][:, lo + o0:lo + o0 + nn_],
                        start=(kt == 0), stop=(kt == 1),
                        skip_group_check=(b == 1))
            nc.scalar.activation(out=x2q[:, 0:682], in_=ps[:, 0:682],
                                 func=Act.Identity, bias=b2p, scale=1.0)
            nc.scalar.activation(out=x2q[:, 682:1178], in_=ps[:, 682:1178],
                                 func=Act.Identity, bias=b2p, scale=1.0)
            nc.scalar.activation(out=x2q[0:64, 1178:1240],
                                 in_=ps[0:64, 1178:1240],
                                 func=Act.Identity, bias=scals[0:64, 5:6],
                                 scale=1.0)

            # x1q: partition block b = x1 at query rows 2t+b.
            ps1 = pshead.tile([128, NP2 * W], f32, tag="psx1")
            for b in range(2):
                for kt in range(2):
                    ob = obn[kt][:].rearrange("p (r w) -> p r w", w=WP)
                    rhs = bass.AP(
                        tensor=ob.tensor,
                        offset=ob.offset + (PAD + b) * WP + PAD,
                        ap=[ob.ap[0], [2 * WP, NP2], [1, W]])
                    nc.tensor.matmul(
                        ps1[64 * b:64 * b + 64, :],
                        w1s[:, kt, :], rhs,
                        start=(kt == 0), stop=(kt == 1),
                        skip_group_check=(b == 1))
            nc.scalar.activation(
                out=x1q[:].rearrange("p a b -> p (a b)"), in_=ps1[:],
                func=Act.Identity, bias=b1p, scale=1.0)

            # x3q: s-split rep-major channels over the full padded region.
            for ot in range(2):
                ps3 = pshead.tile([128, 1240], f32, tag="psx3", name=f"psx3_{ot}")
                for (o0, n) in ccuts:
                    for kt in range(2):
                        nc.tensor.matmul(
                            ps3[:, o0:o0 + n], w3s[:, kt, ot, :],
                            obn[kt][:, o0:o0 + n],
                            start=(kt == 0), stop=(kt == 1))
                nc.scalar.activation(out=x3q[:, ot, :], in_=ps3[:], func=Act.Copy)

        rsubpp = stack.enter_context(tc.tile_pool(name="rsubpp", bufs=2))
        ysballp = stack.enter_context(tc.tile_pool(name="ysballp", bufs=1))
        ysball = ysballp.tile([128, 2, NP2, 2, W], f32, tag="ysball")
        featp = stack.enter_context(tc.tile_pool(name="featp", bufs=3))
        h2p = stack.enter_context(tc.tile_pool(name="h2p", bufs=1))
        e4p = stack.enter_context(tc.tile_pool(name="e4p", bufs=2))
        prodp = stack.enter_context(tc.tile_pool(name="prodp", bufs=1))
        zinp = stack.enter_context(tc.tile_pool(name="zinp", bufs=2))
        smallp = stack.enter_context(tc.tile_pool(name="smallp", bufs=2))
        ps1p = stack.enter_context(tc.tile_pool(name="ps1p", bufs=2, space="PSUM"))
        ps2p = stack.enter_context(tc.tile_pool(name="ps2p", bufs=2, space="PSUM"))
        paggp = stack.enter_context(tc.tile_pool(name="paggp", bufs=1, space="PSUM"))

        chunk_state = {}

        MB1 = 512     # mm1 psum block cols
        MB2 = 512     # mm2 psum block cols per parity (tile is 2x this)

        def phase1(ci):
            (t0, np_) = CHUNKS[ci]
            nq2 = np_ * W
            vc2 = K2 * nq2

            rsub = rsubpp.tile([4, K2, np_, W], f16, tag="rsub", name=f"rsub{ci}")
            nc.sync.dma_start(out=rsub[:], in_=rsubp_d[:, :, t0:t0 + np_, :])

            feat = featp.tile([128, K2, nq2], f16, tag="feat", name=f"feat{ci}")
            # feat[p, di*7+dj, t, w] = x1q[p, t0+t, w] - x2q[p, 2(t0+t)+di, dj+w]
            x1v = x1q[:]
            x2v = x2q[:]
            fv = feat[:]
            for di in range(K):
                x1w = bass.AP(
                    tensor=x1v.tensor, offset=x1v.offset + t0 * W,
                    ap=[x1v.ap[0], [0, K], [W, np_], [1, W]])
                x2w = bass.AP(
                    tensor=x2v.tensor, offset=x2v.offset + (2 * t0 + di) * WP,
                    ap=[x2v.ap[0], [1, K], [2 * WP, np_], [1, W]])
                outw = bass.AP(
                    tensor=fv.tensor, offset=fv.offset + di * K * nq2,
                    ap=[fv.ap[0], [nq2, K], [W, np_], [1, W]])
                eng = nc.gpsimd if di >= 5 else nc.vector
                eng.tensor_tensor(out=outw, in0=x1w, in1=x2w,
                                  op=Alu.subtract)
            featf = feat[:].rearrange("p a b -> p (a b)")
            nc.vector.tensor_scalar_max(out=featf, in0=featf, scalar1=0.0)

            h2 = h2p.tile([128, vc2], f16, tag="h2")
            e4 = e4p.tile([128, 2, K2, nq2], f16, tag="e4", name=f"e4{ci}")
            rsf = rsub[:].rearrange("p a b c -> p (a b c)")
            del rsub
            for j0 in range(0, vc2, MB1):
                n = min(MB1, vc2 - j0)
                psa = ps1p.tile([128, MB1], f32, tag="ps1")
                for s in range(0, n, 512):
                    sn = min(512, n - s)
                    nc.tensor.matmul(psa[:, s:s + sn], cw1s,
                                     featf[:, j0 + s:j0 + s + sn],
                                     start=True, stop=False)
                    nc.tensor.matmul(psa[:, s:s + sn], posws,
                                     rsf[:, j0 + s:j0 + s + sn],
                                     start=False, stop=True)
                nc.scalar.activation(out=h2[:, j0:j0 + n], in_=psa[:, :n],
                                     func=Act.Relu, bias=b2f, scale=1.0)
            for j0 in range(0, vc2, MB2):
                n = min(MB2, vc2 - j0)
                psb = ps2p.tile([128, 2 * MB2], f32, tag="ps2")
                for par in range(2):
                    nc.tensor.matmul(
                        psb[:, par * MB2:par * MB2 + n],
                        cw2s[64 * par:64 * par + 64, :],
                        h2[64 * par:64 * par + 64, j0:j0 + n],
                        start=True, stop=True)
                e4w = bass.AP(
                    tensor=e4.tensor, offset=e4[:].offset + j0,
                    ap=[e4[:].ap[0], [K2 * nq2, 2], [1, n]])
                psbv = psb[:]
                psin = bass.AP(tensor=psb.tensor, offset=psbv.offset,
                               ap=[psbv.ap[0], [MB2, 2], [1, n]])
                nc.scalar.activation(out=e4w, in_=psin,
                                     func=Act.Exp, bias=cb2r, scale=1.0)
            chunk_state[ci] = (e4,)

        def phase2(ci):
            (t0, np_) = CHUNKS[ci]
            nq2 = np_ * W
            (e4,) = chunk_state[ci]

            # normalizer: tree-sum taps on the (already head-replicated) e4
            zt = zinp.tile([128, 2, 24, nq2], f16, tag="zt", name=f"zt{ci}")
            nc.vector.tensor_tensor(
                out=zt[:], in0=e4[:, :, 0:24, :], in1=e4[:, :, 24:48, :],
                op=Alu.add)
            for (a, b_, n) in [(0, 12, 12), (0, 6, 6), (0, 3, 3)]:
                nc.vector.tensor_tensor(
                    out=zt[:, :, a:a + n, :], in0=zt[:, :, a:a + n, :],
                    in1=zt[:, :, b_:b_ + n, :], op=Alu.add)
            for b_ in (1, 2):
                nc.vector.tensor_tensor(
                    out=zt[:, :, 0, :], in0=zt[:, :, 0, :],
                    in1=zt[:, :, b_, :], op=Alu.add)
            nc.vector.tensor_tensor(
                out=zt[:, :, 0, :], in0=zt[:, :, 0, :],
                in1=e4[:, :, 48, :], op=Alu.add)
            rz = smallp.tile([128, 2, nq2], f16, tag="rz", name=f"rz{ci}")
            with nc.allow_low_precision(reason="fp16 softmax reciprocal"):
                nc.vector.reciprocal(
                    out=rz[:], in_=zt[:, :, 0, :])

            for par in range(2):
                e4v = e4[:, par, :, :]
                prodt = prodp.tile([128, 2, K2, nq2], f16, tag=f"prod{par}",
                                   name=f"prod{par}")
                pv = prodt[:]
                pnum = paggp.tile([128, 2 * nq2], f32, tag="pnum",
                                  name=f"pnum{par}")
                POOL_DI = {0: (5, 6), 1: (5, 6)}   # ot -> pool di set
                for ot in range(2):
                    pool_di = POOL_DI[ot]
                    for di in list(pool_di) + [d for d in range(K)
                                               if d not in pool_di]:
                        e4w = bass.AP(
                            tensor=e4.tensor,
                            offset=e4v.offset + di * K * nq2,
                            ap=[e4v.ap[0], [nq2, K], [W, np_], [1, W]])
                        x3w = bass.AP(
                            tensor=x3q.tensor,
                            offset=x3q[:].offset + ot * ROWS * WP
                            + (2 * t0 + par + di) * WP,
                            ap=[x3q[:].ap[0], [1, K], [2 * WP, np_], [1, W]])
                        outw = bass.AP(
                            tensor=prodt.tensor,
                            offset=pv.offset + ot * K2 * nq2 + di * K * nq2,
                            ap=[pv.ap[0], [nq2, K], [W, np_], [1, W]])
                        eng = nc.gpsimd if di in pool_di else nc.vector
                        eng.tensor_tensor(out=outw, in0=e4w, in1=x3w,
                                          op=Alu.mult)
                    # numerator: PE identity-accumulation over taps, per ot.
                    # DVE-produced taps first; slower GPSIMD taps last.
                    korder = ([k for k in range(K2) if k // K not in pool_di]
                              + [k for k in range(K2) if k // K in pool_di])
                    for i, k in enumerate(korder):
                        nc.tensor.matmul(
                            pnum[:, ot * nq2:(ot + 1) * nq2], idents,
                            pv[:, ot, k, :],
                            start=(i == 0), stop=(i == K2 - 1),
                            skip_group_check=(ot == 1))

                # divide + bn2/relu + wc + residual
                ob = smallp.tile([128, 2, nq2], f32, tag="ob", name=f"ob{par}")
                rzw = bass.AP(
                    tensor=rz.tensor, offset=rz[:].offset + par * nq2,
                    ap=[rz[:].ap[0], [0, 2], [1, nq2]])
                nc.vector.tensor_tensor(
                    out=ob[:], in0=pnum[:].rearrange("p (a b) -> p a b", a=2),
                    in1=rzw, op=Alu.mult)
                ob2 = smallp.tile([128, 2, nq2], bf16, tag="ob2",
                                  name=f"ob2{par}")
                for ot in range(2):
                    nc.vector.tensor_scalar(
                        out=ob2[:, ot, :], in0=ob[:, ot, :],
                        scalar1=a3p[ot], scalar2=b3fp[ot],
                        op0=Alu.mult, op1=Alu.add)
                    nc.vector.tensor_scalar_max(
                        out=ob2[:, ot, :], in0=ob2[:, ot, :], scalar1=0.0)
                for oo in range(2):
                    psw = paggp.tile([128, 224], f32, tag="pz",
                                    name=f"pswc{par}{oo}")
                    for kt in range(2):
                        nc.tensor.matmul(psw[:, :nq2], wcs[:, kt, oo, :],
                                         ob2[:, kt, :],
                                         start=(kt == 0), stop=(kt == 1))
                    xi = bass.AP(
                        tensor=xps[oo].tensor,
                        offset=xps[oo][:].offset
                        + (PAD + 2 * t0 + par) * WP + PAD,
                        ap=[xps[oo][:].ap[0], [2 * WP, np_], [1, W]])
                    nc.vector.scalar_tensor_tensor(
                        out=ysball[:, oo, t0:t0 + np_, par, :],
                        in0=psw[:, :nq2].rearrange("p (r w) -> p r w", w=W),
                        scalar=bcb[oo], in1=xi,
                        op0=Alu.add, op1=Alu.add)

        def flush_y(ci):
            (t0, np_) = CHUNKS[ci]
            for oo in range(2):
                nc.sync.dma_start(
                    out=y_d[oo][:, 2 * t0:2 * t0 + 2 * np_, :],
                    in_=ysball[:, oo, t0:t0 + np_, :, :].rearrange(
                        "p t q w -> p (t q) w"))

        phase1(0)
        for ci in range(1, len(CHUNKS)):
            phase1(ci)
            phase2(ci - 1)
            flush_y(ci - 1)
        phase2(len(CHUNKS) - 1)
        flush_y(len(CHUNKS) - 1)

    nc.compile()
    _BUILD_CACHE["nc"] = nc
    return nc


def _host_prep(inputs):
    f = {k: np.asarray(v, np.float32) for k, v in inputs.items()}

    def fold(n):
        a = f[n + "_g"] / np.sqrt(f[n + "_rv"] + EPS)
        return a, f[n + "_b"] - f[n + "_rm"] * a

    a1, b1f = fold("bn1")
    ac, bc1 = fold("cwbn1")
    a2, b2f = fold("cwbn2")
    a3, b3f = fold("bn2")

    W1p = ac[:64, None] * f["w1"]
    b1p = ac[:64] * f["b1"] + bc1[:64]
    W2p = ac[:64, None] * f["w2"]
    b2p = ac[:64] * f["b2"]
    cw1p = a2[:, None] * f["cw1"]          # [64, 66]

    perm = _perm_channels()
    w3p = f["w3"][perm]
    a3p = a3[perm]
    b3fp = b3f[perm]
    rep = np.arange(128) % 32
    cw2r = f["cw2"][rep]
    cb2r = f["cb2"][rep]

    locw = np.tile(np.linspace(-1.0, 1.0, W, dtype=np.float32)[None, :], (H, 1))
    loch = np.tile(np.linspace(-1.0, 1.0, H, dtype=np.float32)[:, None], (1, W))
    loc = np.stack([locw, loch], 0)
    p = np.einsum("chw,oc->ohw", loc, f["pw"]) + f["pb"][:, None, None]
    pp = np.pad(p, ((0, 0), (PAD, PAD), (PAD, PAD)), mode="reflect")
    pu = np.stack([pp[:, i:i + H, j:j + W] for i in range(K) for j in range(K)], 1)
    subp = p[:, None] - pu
    rsubp = np.maximum(ac[64:66, None, None, None] * subp
                       + bc1[64:66, None, None, None], 0).astype(np.float16)

    xpad = np.pad(f["x"], ((0, 0), (0, 0), (PAD, PAD), (PAD, PAD)), mode="reflect")

    w1T = np.ascontiguousarray(W1p.T).reshape(2, 128, 64).copy()
    w2T = np.ascontiguousarray(W2p.T).reshape(2, 128, 64).copy()
    w3T = np.empty((2, 128, 2, 128), np.float32)
    wcT = np.empty((2, 128, 2, 128), np.float32)
    wc_perm = f["wc"][:, perm]
    for kt in range(2):
        for ot in range(2):
            w3T[kt, :, ot, :] = w3p[ot * 128:(ot + 1) * 128,
                                    kt * 128:(kt + 1) * 128].T
            wcT[kt, :, ot, :] = wc_perm[ot * 128:(ot + 1) * 128,
                                        kt * 128:(kt + 1) * 128].T

    cw1T = np.zeros((128, 128), np.float32)
    cw1T[0:64, 0:64] = cw1p[:, 0:64].T
    cw1T[64:128, 64:128] = cw1p[:, 0:64].T
    poswT = np.zeros((4, 128), np.float32)
    poswT[0:2, 0:64] = cw1p[:, 64:66].T
    poswT[2:4, 64:128] = cw1p[:, 64:66].T
    cw2T = np.zeros((128, 128), np.float32)
    cw2T[0:64, :] = cw2r.T
    cw2T[64:128, :] = cw2r.T


    scal = np.zeros((128, 14), np.float32)
    scal[:, 0] = a1[:128]; scal[:, 1] = a1[128:]
    scal[:, 2] = b1f[:128]; scal[:, 3] = b1f[128:]
    scal[:64, 4] = b1p; scal[64:, 4] = b1p
    scal[:64, 5] = b2p; scal[64:, 5] = b2p
    scal[:64, 6] = b2f; scal[64:, 6] = b2f
    scal[:, 7] = cb2r
    scal[:, 8] = a3p[:128]; scal[:, 9] = a3p[128:]
    scal[:, 10] = b3fp[:128]; scal[:, 11] = b3fp[128:]
    scal[:, 12] = f["bc"][:128]; scal[:, 13] = f["bc"][128:]

    wb = np.zeros((128, 1280), np.float32)
    wb[:, 0:64] = w1T[0]; wb[:, 64:128] = w1T[1]
    wb[:, 128:192] = w2T[0]; wb[:, 192:256] = w2T[1]
    wb[:, 256:768] = w3T.transpose(1, 0, 2, 3).reshape(128, 512)
    wb[:, 768:1280] = wcT.transpose(1, 0, 2, 3).reshape(128, 512)
    fbb = np.zeros((128, 640), np.float32)
    fbb[:, 0:128] = cw1T
    fbb[:, 128:256] = cw2T
    fbb[0:4, 256:384] = poswT
    fbb[:, 384:512] = np.eye(128, dtype=np.float32)
    for m in range(4):
        fbb[np.arange(32), 512 + 32 * m + np.arange(32)] = 1.0
    shared = dict(wb=wb.astype(bf16_np), fb=fbb.astype(np.float16),
                  scal=scal)
    in_maps = []
    for core in range(8):
        b, i = divmod(core, 4)
        r0 = RB * i
        m = dict(shared)
        m["xp"] = np.ascontiguousarray(
            xpad[b].reshape(2, 128, H + 2 * PAD, WP)[:, :, r0:r0 + ROWS, :])
        rs = rsubp[:, :, r0:r0 + RB, :]           # [2, 49, 14, 56]
        rs4 = np.empty((4, K2, NP2, W), np.float16)
        rs4[0:2] = rs[:, :, 0::2, :]
        rs4[2:4] = rs[:, :, 1::2, :]
        m["rsubp"] = np.ascontiguousarray(rs4)
        in_maps.append(m)
    return in_maps


def kernel(**inputs):
    from concourse.bass_utils import run_bass_kernel_spmd
    nc = _build_program()
    in_maps = _host_prep(inputs)
    res = run_bass_kernel_spmd(nc, in_maps, core_ids=list(range(8)))
    global LAST_RESULTS
    LAST_RESULTS = res
    y = np.zeros((B, C, H, W), np.float32)
    for core in range(8):
        b, i = divmod(core, 4)
        yc = res.results[core]["y"]
        y[b, :, RB * i:RB * (i + 1), :] = yc.reshape(C, RB, W)
    return y
